# revision 1
# baseline (speedup 1.0000x reference)
"""DCN block kernel for Trainium2 (8 NeuronCores, data-parallel over batch).

Math (per batch b, plane c):
  z   = conv3x3(x, w_off) + b_off                  (64 offset logits)
  d   = sigmoid(z) - 0.5   in (-.5, .5)            (pixel displacement)
  sample at (r - dy, c - dx) bilinear w/ reflect   (|d| < .5 => 3x3 support!)
  y   = conv3x3(sampled, w_dcn) + b_dcn

Because |d| < 0.5 the bilinear gather only touches the 3x3 neighborhood, so it
is computed gather-free as
  H(sigma)  = x + dxt*AR + |dxt|*BR     (AR = x(c-1)-x(c+1), BR = x(c-1)+x(c+1)-2x)
  out = H0 + dyt*(Hm-Hp) + |dyt|*(Hm+Hp-2H0)
with dxt = d/2. With reflect-consistent fixups at image border rows/cols this
is exact.

Layout: 4 image row-quarters stacked on partition groups [4 x 32ch]; convs run
as 4 concurrent row-tiled matmul streams (tile_position), K=32, 9 taps
accumulating in PSUM; elementwise sampling runs on [128, fd] bf16 tiles.
"""

import math
from contextlib import ExitStack

import ml_dtypes
import numpy as np

import concourse.bacc as bacc
import concourse.bass as bass
import concourse.mybir as mybir
import concourse.tile as tile

BF16 = mybir.dt.bfloat16
F32 = mybir.dt.float32
AF = mybir.ActivationFunctionType
OP = mybir.AluOpType

N_CORES = 8
C = 32          # input/output channels per plane set
OC2 = 64        # offset logits (2 per plane)


class Cfg:
    def __init__(self, H=384, nr=8):
        self.H = H
        self.W = H
        self.WP = self.W + 2          # padded row: [pad, 0..W-1, pad]
        self.QH = H // 4              # rows per quarter
        assert self.QH % nr == 0
        self.nr = nr                  # output rows per quarter per slab
        self.nslab = self.QH // nr


def _f(ap):
    """Flatten free dims of a 3d tile AP to [P, fd]."""
    return ap.rearrange("p a b -> p (a b)")


def build_nc(cfg: Cfg, finalize=True):
    nc = bacc.Bacc()
    H, W, WP, nr = cfg.H, cfg.W, cfg.WP, cfg.nr

    x_in = nc.declare_dram_parameter("x", [C, H + 4, W], BF16, isOutput=False)
    woff_in = nc.declare_dram_parameter("woff", [128, 9 * OC2], BF16, isOutput=False)
    wdcn_in = nc.declare_dram_parameter("wdcn", [128, 9 * C], BF16, isOutput=False)
    boff_in = nc.declare_dram_parameter("boff", [128, 1], F32, isOutput=False)
    bdcn_in = nc.declare_dram_parameter("bdcn", [128, 1], F32, isOutput=False)
    y_out = nc.declare_dram_parameter("y", [C, H, W], F32, isOutput=True)

    with tile.TileContext(nc) as tc, ExitStack() as ctx:
        fold_sem = ctx.enter_context(nc.semaphore("fold_sem"))
        fold_cnt = [0]
        store_sem = ctx.enter_context(nc.semaphore("store_sem"))
        store_cnt = [0]
        consts = ctx.enter_context(tc.tile_pool(name="consts", bufs=1))
        xpool = ctx.enter_context(tc.tile_pool(name="xp", bufs=1))
        abpool = ctx.enter_context(tc.tile_pool(name="abp", bufs=1))
        spool = ctx.enter_context(tc.tile_pool(name="sp", bufs=1))
        mpool = ctx.enter_context(tc.tile_pool(name="mp", bufs=1))
        hpool = ctx.enter_context(tc.tile_pool(name="hp", bufs=1))
        ospool = ctx.enter_context(tc.tile_pool(name="osp", bufs=1))
        ocpool = ctx.enter_context(tc.tile_pool(name="ocp", bufs=2))
        zpool = ctx.enter_context(tc.tile_pool(name="zp", bufs=2, space="PSUM"))
        opool = ctx.enter_context(tc.tile_pool(name="op", bufs=2, space="PSUM"))

        WOFF = consts.tile([128, 9, OC2], BF16)
        nc.sync.dma_start(out=_f(WOFF), in_=woff_in[:])
        WDCN = consts.tile([128, 9, C], BF16)
        nc.sync.dma_start(out=_f(WDCN), in_=wdcn_in[:])
        BOFF = consts.tile([128, 1], F32)
        nc.sync.dma_start(out=BOFF[:], in_=boff_in[:])
        BDCN = consts.tile([128, 1], F32)
        nc.sync.dma_start(out=BDCN[:], in_=bdcn_in[:])
        WU = consts.tile([128, 1], F32)
        nc.scalar.activation(out=WU[:], in_=BOFF[:], func=AF.Identity,
                             bias=BDCN[:], scale=1.0)
        NEG25 = consts.tile([128, 1], F32)
        nc.vector.memset(NEG25[:], -0.25)

        nh = nr + 2   # z/s/H/OS rows: [r0-1, r0+nr+1)
        nx = nr + 4   # x rows:       [r0-2, r0+nr+2)
        # persistent x slabs (ping-pong): pre-zero once -> pad cols and
        # first-slab halo rows stay zero forever
        XS_pp = []
        for sl in range(2):
            XSz = xpool.tile([128, nx, WP], BF16, tag=f"xs{sl}", name=f"xsz{sl}")
            nc.vector.memset(_f(XSz), 0.0)
            XS_pp.append(XSz)
        # persistent sigmoid-pair tiles: pad cols pre-zeroed once
        SP = []
        for p in range(2):
            SPp = spool.tile([128, nh, WP], BF16, tag=f"sp{p}", name=f"spp{p}")
            nc.vector.memset(SPp[:, :, 0:WP:W + 1], 0.0)
            SP.append(SPp)

        for it in range(cfg.nslab):
            r0 = it * nr

            # ---- load x slab: 4 quarters stacked on partition groups ----
            XS = XS_pp[it % 2]
            for g in range(4):
                i0 = cfg.QH * g + r0   # row in the padded-x frame
                nc.sync.dma_start(
                    out=XS[32 * g:32 * g + 32, :, 1:W + 1],
                    in_=x_in[:, i0:i0 + nx, :])
            XSf = _f(XS[:])

            # ---- conv_off + sigmoid, pairs (q0,q1)->ztA, (q2,q3)->ztB ----
            for zi in range(nh):
                zts = [zpool.tile([128, 512], F32, tag=f"z{p}", name=f"zt{p}")
                       for p in range(2)]
                for t in range(9):
                    kh, kw = t // 3, t % 3
                    base = (zi + kh) * WP + kw
                    for p in range(2):
                        for gq in range(2):
                            g = 2 * p + gq
                            nc.tensor.matmul(
                                zts[p][64 * gq:64 * gq + 64, 1:W + 1],
                                lhsT=WOFF[32 * g:32 * g + 32, t, :],
                                rhs=XSf[32 * g:32 * g + 32, base:base + W],
                                start=(t == 0), stop=(t == 8),
                                tile_position=(32 * g, 64 * gq),
                                skip_group_check=True)
                for p in range(2):
                    nc.scalar.activation(
                        out=SP[p][:, zi, 1:W + 1], in_=zts[p][:, 1:W + 1],
                        func=AF.Sigmoid, bias=BOFF[:], scale=1.0)

            # ---- fold s into quarter-stacked SX/SY (8 sbuf->sbuf DMAs) ----
            SX = spool.tile([128, nh, WP], BF16, tag="sx")
            SY = spool.tile([128, nh, WP], BF16, tag="sy")
            with tc.tile_critical():
                for g in range(4):
                    p, gq = g // 2, g % 2
                    nc.gpsimd.dma_start(
                        out=_f(SX[32 * g:32 * g + 32]),
                        in_=_f(SP[p][64 * gq:64 * gq + 32])).then_inc(fold_sem, 16)
                    nc.gpsimd.dma_start(
                        out=_f(SY[32 * g:32 * g + 32]),
                        in_=_f(SP[p][64 * gq + 32:64 * gq + 64])).then_inc(fold_sem, 16)
                fold_cnt[0] += 128
                nc.gpsimd.wait_ge(fold_sem, fold_cnt[0])

            # ---- displacement maps: dxt = s/2 - 1/4, |dxt| ----
            DX = mpool.tile([128, nh, WP], BF16, tag="dx")
            nc.vector.tensor_scalar(_f(DX), _f(SX), 0.5, -0.25, OP.mult, OP.add)
            ADX = mpool.tile([128, nh, WP], BF16, tag="adx")
            nc.scalar.activation(out=_f(ADX), in_=_f(SX), func=AF.Abs,
                                 bias=NEG25[:], scale=0.5)

            # ---- column diff images on x geometry ----
            Lx = nx * WP
            AR = abpool.tile([128, nx, WP], BF16, tag="ar")
            ARf = _f(AR)
            nc.vector.tensor_tensor(
                ARf[:, 1:Lx - 1], XSf[:, 0:Lx - 2], XSf[:, 2:Lx], OP.subtract)
            BR0 = abpool.tile([128, nx, WP], BF16, tag="br0")
            BR0f = _f(BR0)
            nc.gpsimd.tensor_tensor(
                BR0f[:, 1:Lx - 1], XSf[:, 0:Lx - 2], XSf[:, 2:Lx], OP.add)
            BR = abpool.tile([128, nx, WP], BF16, tag="br")
            BRf = _f(BR)
            nc.vector.scalar_tensor_tensor(
                BRf[:, 1:Lx - 1], in0=XSf[:, 1:Lx - 1], scalar=-2.0,
                in1=BR0f[:, 1:Lx - 1], op0=OP.mult, op1=OP.add)
            for tl in (ARf, BRf):
                nc.vector.memset(tl[:, 0:1], 0.0)
                nc.vector.memset(tl[:, Lx - 1:Lx], 0.0)
            # reflect fixups at image cols 0 / W-1 (padded cols 1 / W)
            nc.vector.memset(AR[:, :, 1], 0.0)
            nc.vector.memset(AR[:, :, W], 0.0)
            nc.vector.tensor_tensor(
                BR[:, :, 1], BR[:, :, 1], XS[:, :, 2], OP.add)
            nc.vector.tensor_tensor(
                BR[:, :, W], BR[:, :, W], XS[:, :, W - 1], OP.add)

            # ---- horizontal interps H(-1), H(0), H(+1) ----
            Lh = nh * WP
            DXf, ADXf = _f(DX), _f(ADX)
            Hs = []
            for dr in (-1, 0, 1):
                off = (1 + dr) * WP
                T1 = hpool.tile([128, nh, WP], BF16, tag="ht1")
                nc.vector.tensor_tensor(
                    _f(T1), DXf, ARf[:, off:off + Lh], OP.mult)
                T2 = hpool.tile([128, nh, WP], BF16, tag="ht2")
                nc.vector.tensor_tensor(
                    _f(T2), ADXf, BRf[:, off:off + Lh], OP.mult)
                Hd = hpool.tile([128, nh, WP], BF16, tag=f"h{dr}")
                nc.vector.tensor_tensor(
                    _f(Hd), _f(T1), XSf[:, off:off + Lh], OP.add)
                nc.vector.tensor_tensor(_f(Hd), _f(Hd), _f(T2), OP.add)
                Hs.append(Hd)
            Hm, H0, Hp = Hs

            # ---- vertical combine ----
            AH = hpool.tile([128, nh, WP], BF16, tag="ah")
            nc.vector.tensor_tensor(_f(AH), _f(Hm), _f(Hp), OP.subtract)
            BH0 = hpool.tile([128, nh, WP], BF16, tag="ht2")
            nc.vector.tensor_tensor(_f(BH0), _f(Hm), _f(Hp), OP.add)
            BH = hpool.tile([128, nh, WP], BF16, tag="bh")
            nc.vector.scalar_tensor_tensor(
                _f(BH), in0=_f(H0), scalar=-2.0, in1=_f(BH0),
                op0=OP.mult, op1=OP.add)
            # y displacement maps (reuse the dx/adx slots)
            DY = mpool.tile([128, nh, WP], BF16, tag="dx")
            nc.vector.tensor_scalar(_f(DY), _f(SY), 0.5, -0.25, OP.mult, OP.add)
            ADY = mpool.tile([128, nh, WP], BF16, tag="adx")
            nc.scalar.activation(out=_f(ADY), in_=_f(SY), func=AF.Abs,
                                 bias=NEG25[:], scale=0.5)
            # reflect fixups at image rows 0 / H-1 (Hm/Hp read zero rows there)
            if it == 0:
                nc.vector.memset(_f(AH[0:32, 1:2, :]), 0.0)
                nc.vector.tensor_tensor(
                    _f(BH[0:32, 1:2, :]), _f(BH[0:32, 1:2, :]),
                    _f(Hp[0:32, 1:2, :]), OP.add)
            if it == cfg.nslab - 1:
                nc.vector.memset(_f(AH[96:128, nr:nr + 1, :]), 0.0)
                nc.vector.tensor_tensor(
                    _f(BH[96:128, nr:nr + 1, :]), _f(BH[96:128, nr:nr + 1, :]),
                    _f(Hm[96:128, nr:nr + 1, :]), OP.add)

            OS = ospool.tile([128, nh, WP], BF16)
            T3 = hpool.tile([128, nh, WP], BF16, tag="ht1")
            nc.vector.tensor_tensor(_f(T3), _f(DY), _f(AH), OP.mult)
            T4 = hpool.tile([128, nh, WP], BF16, tag="ht2")
            nc.vector.tensor_tensor(_f(T4), _f(ADY), _f(BH), OP.mult)
            nc.vector.tensor_tensor(_f(OS), _f(H0), _f(T3), OP.add)
            nc.vector.tensor_tensor(_f(OS), _f(OS), _f(T4), OP.add)
            # sampled outside the image is 0 for the final conv zero-padding
            nc.vector.memset(OS[:, :, 0:WP:W + 1], 0.0)
            if it == 0:
                nc.vector.memset(_f(OS[0:32, 0:1, :]), 0.0)
            if it == cfg.nslab - 1:
                nc.vector.memset(_f(OS[96:128, nr + 1:nr + 2, :]), 0.0)
            OSf = _f(OS)

            # ---- conv_dcn + bias + store (4-row output chunks) ----
            OROWS = 4
            for oc_i in range(nr // OROWS):
                OC = ocpool.tile([128, OROWS, WP], F32)
                for oj in range(OROWS):
                    oi = oc_i * OROWS + oj
                    ot = opool.tile([128, 512], F32)
                    for t in range(9):
                        kh, kw = t // 3, t % 3
                        base = (oi + kh) * WP + kw
                        for g in range(4):
                            nc.tensor.matmul(
                                ot[32 * g:32 * g + 32, 1:W + 1],
                                lhsT=WDCN[32 * g:32 * g + 32, t, :],
                                rhs=OSf[32 * g:32 * g + 32, base:base + W],
                                start=(t == 0), stop=(t == 8),
                                tile_position=(32 * g, 32 * g),
                                skip_group_check=True)
                    nc.scalar.activation(
                        out=OC[:, oj, 1:W + 1], in_=ot[:, 1:W + 1],
                        func=AF.Identity, bias=BDCN[:], scale=1.0)
                with tc.tile_critical():
                    for g in range(4):
                        rr = cfg.QH * g + r0 + oc_i * OROWS
                        nc.gpsimd.dma_start(
                            out=y_out[:, rr:rr + OROWS, :],
                            in_=OC[32 * g:32 * g + 32, :, 1:W + 1]
                        ).then_inc(store_sem, 16)
                    store_cnt[0] += 64
                    nc.gpsimd.wait_ge(store_sem, store_cnt[0])
    if finalize:
        nc.finalize()
    return nc


def prep_weights(w_off, b_off, w_dcn, b_dcn):
    """Host-side packing of conv weights into lhsT tiles, replicated x4."""
    perm = np.concatenate([np.arange(0, 2 * C, 2), np.arange(1, 2 * C, 2)])
    # WOFF[32g+ci, kh*3+kw, m] = w_off[perm[m], ci, kh, kw]
    wo = w_off[perm].astype(np.float32)            # [64, C, 3, 3]
    wo = wo.transpose(1, 2, 3, 0).reshape(C, 9, OC2)   # [ci, tap, m]
    woff = np.tile(wo, (4, 1, 1)).reshape(128, 9 * OC2)
    wd = w_dcn.astype(np.float32).transpose(1, 2, 3, 0).reshape(C, 9, C)
    wdcn = np.tile(wd, (4, 1, 1)).reshape(128, 9 * C)
    boff = np.tile(b_off[perm].astype(np.float32), 2).reshape(128, 1)
    bdcn = np.tile(b_dcn.astype(np.float32), 4).reshape(128, 1)
    return {
        "woff": woff.astype(ml_dtypes.bfloat16),
        "wdcn": wdcn.astype(ml_dtypes.bfloat16),
        "boff": boff.astype(np.float32),
        "bdcn": bdcn.astype(np.float32),
    }


_NC_CACHE = {}


def _get_nc(cfg_key):
    if cfg_key not in _NC_CACHE:
        _NC_CACHE[cfg_key] = build_nc(Cfg(H=cfg_key[0], nr=cfg_key[1]))
    return _NC_CACHE[cfg_key]


def _run(x, w_off, b_off, w_dcn, b_dcn, **spmd_kwargs):
    from concourse.bass_utils import run_bass_kernel_spmd

    B = x.shape[0]
    H = x.shape[2]
    assert x.shape == (B, C, H, H) and B == N_CORES
    nc = _get_nc((H, 8))
    w = prep_weights(np.asarray(w_off), np.asarray(b_off),
                     np.asarray(w_dcn), np.asarray(b_dcn))
    in_maps = []
    for b in range(B):
        m = dict(w)
        xb = np.asarray(x[b]).astype(ml_dtypes.bfloat16)
        m["x"] = np.pad(xb, ((0, 0), (2, 2), (0, 0)))
        in_maps.append(m)
    return run_bass_kernel_spmd(nc, in_maps, list(range(N_CORES)), **spmd_kwargs)


def kernel(x, w_off, b_off, w_dcn, b_dcn):
    res = _run(x, w_off, b_off, w_dcn, b_dcn)
    out = np.stack([res.results[i]["y"] for i in range(N_CORES)], axis=0)
    return out.astype(np.float32)



# revision 6
# speedup vs baseline: 1.3349x; 1.3349x over previous
"""DCN block kernel for Trainium2 (8 NeuronCores, data-parallel over batch).

Math (per batch b, plane c):
  z   = conv3x3(x, w_off) + b_off                  (64 offset logits)
  s   = sigmoid(z); d = s - 0.5 in (-.5,.5)        (pixel displacement)
  sample at (r - dy, c - dx) bilinear w/ reflect   (|d| < .5 => 3x3 support)
  y   = conv3x3(sampled, w_dcn) + b_dcn

Gather-free sampling via relu-split weights (all DVE ops run in 2x/4x mode):
  A  = relu(sx-.5)  B = relu(.5-sx)   (horizontal taps)
  H_dr = x_dr + A*DL_dr + B*DR_dr     DL = x(c-1)-x(c), DR = x(c+1)-x(c)
  OS = H0 + Av*(Hm-H0) + Bv*(Hp-H0)   (vertical combine)
Reflect at borders: column fixups on DL/DR; row fixups swap Hm/Hp terms at
the global top/bottom rows.

Layout: 4 image row-quarters stacked on partition groups [4 x 32ch]; convs
run as 4 concurrent row-tiled matmul streams (tile_position), 9 taps
accumulating in PSUM.  The offset stage for slab i+1 is issued BEFORE the
sampling stage of slab i so PE/scalar work overlaps the vector pipeline.
"""

from contextlib import ExitStack

import ml_dtypes
import numpy as np

import concourse.bacc as bacc
import concourse.bass as bass
import concourse.mybir as mybir
import concourse.tile as tile

BF16 = mybir.dt.bfloat16
F32 = mybir.dt.float32
AF = mybir.ActivationFunctionType
OP = mybir.AluOpType

N_CORES = 8
C = 32          # input/output channels per plane set
OC2 = 64        # offset logits (2 per plane)


class Cfg:
    def __init__(self, H=384, nr=8):
        self.H = H
        self.W = H
        self.WP = self.W + 2          # padded row: [pad, 0..W-1, pad]
        self.QH = H // 4              # rows per quarter
        assert self.QH % nr == 0
        self.nr = nr                  # output rows per quarter per slab
        self.nslab = self.QH // nr


def _f(ap):
    """Flatten free dims of a 3d tile AP to [P, fd]."""
    return ap.rearrange("p a b -> p (a b)")


def build_nc(cfg: Cfg, finalize=True):
    nc = bacc.Bacc()
    H, W, WP, nr = cfg.H, cfg.W, cfg.WP, cfg.nr
    nh = nr + 2   # z/s/H/OS rows: [r0-1, r0+nr+1)
    nx = nr + 4   # x rows:       [r0-2, r0+nr+2)
    nslab = cfg.nslab

    x_in = nc.declare_dram_parameter("x", [C, H + 4, W], BF16, isOutput=False)
    woff_in = nc.declare_dram_parameter("woff", [128, 9 * OC2], BF16, isOutput=False)
    wdcn_in = nc.declare_dram_parameter("wdcn", [128, 9 * C], BF16, isOutput=False)
    boff_in = nc.declare_dram_parameter("boff", [128, 1], F32, isOutput=False)
    bdcn_in = nc.declare_dram_parameter("bdcn", [128, 1], F32, isOutput=False)
    y_out = nc.declare_dram_parameter("y", [C, H, W], F32, isOutput=True)

    with tile.TileContext(nc) as tc, ExitStack() as ctx:
        consts = ctx.enter_context(tc.tile_pool(name="consts", bufs=1))
        xpool = ctx.enter_context(tc.tile_pool(name="xp", bufs=2))
        spool = ctx.enter_context(tc.tile_pool(name="sp", bufs=1))
        sxpool = ctx.enter_context(tc.tile_pool(name="sxp", bufs=2))
        bpool = ctx.enter_context(tc.tile_pool(name="bp", bufs=2))
        dpool = ctx.enter_context(tc.tile_pool(name="dp", bufs=1))
        hpool = ctx.enter_context(tc.tile_pool(name="hp", bufs=1))
        ospool = ctx.enter_context(tc.tile_pool(name="osp", bufs=2))
        ocpool = ctx.enter_context(tc.tile_pool(name="ocp", bufs=2))
        zpool = ctx.enter_context(tc.tile_pool(name="zp", bufs=2, space="PSUM"))
        opool = ctx.enter_context(tc.tile_pool(name="op", bufs=2, space="PSUM"))

        WOFF = consts.tile([128, 9, OC2], BF16)
        nc.sync.dma_start(out=_f(WOFF), in_=woff_in[:])
        WDCN = consts.tile([128, 9, C], BF16)
        nc.sync.dma_start(out=_f(WDCN), in_=wdcn_in[:])
        BOFF = consts.tile([128, 1], F32)
        nc.sync.dma_start(out=BOFF[:], in_=boff_in[:])
        BDCN = consts.tile([128, 1], F32)
        nc.sync.dma_start(out=BDCN[:], in_=bdcn_in[:])
        PHALF = consts.tile([128, 1], F32)
        nc.vector.memset(PHALF[:], 0.5)

        # persistent x slabs: pre-zero once so pad cols stay zero forever
        XS_list = []
        for sl in range(2):
            XSz = xpool.tile([128, nx, WP], BF16, tag="xs", name=f"xsz{sl}")
            nc.vector.memset(_f(XSz), 0.0)
            XS_list.append(XSz)

        state = {}   # per-slab live tiles, keyed (name, slab)

        def load_slab(it):
            if it < 2:
                XS = XS_list[it % 2]
            else:
                XS = xpool.tile([128, nx, WP], BF16, tag="xs", name="xs")
            state["xs", it] = XS
            for g in range(4):
                i0 = cfg.QH * g + it * nr   # row in the padded-x frame
                nc.sync.dma_start(
                    out=XS[32 * g:32 * g + 32, :, 1:W + 1],
                    in_=x_in[:, i0:i0 + nx, :])

        def stage_offset(it):
            """conv_off + sigmoid + fold + B2/Bv2 acts for slab it."""
            XS = state["xs", it]
            XSf = _f(XS[:])
            # conv_off: pairs (q0,q1)->zts[0], (q2,q3)->zts[1]
            SP = [spool.tile([128, nh, WP], BF16, tag=f"sp{p}", name=f"sp{p}")
                  for p in range(2)]
            for zi in range(nh):
                zts = [zpool.tile([128, 512], F32, tag=f"z{p}", name=f"z{p}")
                       for p in range(2)]
                for t in range(9):
                    kh, kw = t // 3, t % 3
                    base = (zi + kh) * WP + kw
                    for p in range(2):
                        for gq in range(2):
                            g = 2 * p + gq
                            nc.tensor.matmul(
                                zts[p][64 * gq:64 * gq + 64, 1:W + 1],
                                lhsT=WOFF[32 * g:32 * g + 32, t, :],
                                rhs=XSf[32 * g:32 * g + 32, base:base + W],
                                start=(t == 0), stop=(t == 8),
                                tile_position=(32 * g, 64 * gq),
                                skip_group_check=True)
                for p in range(2):
                    nc.scalar.activation(
                        out=SP[p][:, zi, 1:W + 1], in_=zts[p][:, 1:W + 1],
                        func=AF.Sigmoid, bias=BOFF[:], scale=1.0)
            # fold s into quarter-stacked SX/SY (8 sbuf->sbuf DMAs)
            SX = sxpool.tile([128, nh, WP], BF16, tag="sx")
            SY = sxpool.tile([128, nh, WP], BF16, tag="sy")
            for g in range(4):
                p, gq = g // 2, g % 2
                nc.sync.dma_start(
                    out=_f(SX[32 * g:32 * g + 32]),
                    in_=_f(SP[p][64 * gq:64 * gq + 32]))
                nc.sync.dma_start(
                    out=_f(SY[32 * g:32 * g + 32]),
                    in_=_f(SP[p][64 * gq + 32:64 * gq + 64]))
            # B maps (scalar engine): B2 = relu(.5-sx), Bv2 = relu(.5-sy)
            B2 = bpool.tile([128, nh, WP], BF16, tag="b2")
            nc.scalar.activation(out=_f(B2), in_=_f(SX), func=AF.Relu,
                                 bias=PHALF[:], scale=-1.0)
            BV2 = bpool.tile([128, nh, WP], BF16, tag="bv2")
            nc.scalar.activation(out=_f(BV2), in_=_f(SY), func=AF.Relu,
                                 bias=PHALF[:], scale=-1.0)
            state["sx", it] = SX
            state["sy", it] = SY
            state["b2", it] = B2
            state["bv2", it] = BV2

        def stage_diff(it):
            """DL/DR difference images on gpsimd (runs a slab ahead)."""
            XS = state["xs", it]
            XSf = _f(XS[:])
            L = nx * WP
            DL = dpool.tile([128, nx, WP], BF16, tag="dl")
            DLf = _f(DL)
            nc.gpsimd.tensor_tensor(
                DLf[:, 1:L], XSf[:, 0:L - 1], XSf[:, 1:L], OP.subtract)
            DR = dpool.tile([128, nx, WP], BF16, tag="dr")
            DRf = _f(DR)
            nc.gpsimd.tensor_tensor(
                DRf[:, 0:L - 1], XSf[:, 1:L], XSf[:, 0:L - 1], OP.subtract)
            # reflect fixups at image cols 0 / W-1 (padded cols 1 / W)
            nc.gpsimd.tensor_tensor(
                DL[:, :, 1], DL[:, :, 1], XS[:, :, 2], OP.add)
            nc.gpsimd.tensor_tensor(
                DR[:, :, W], DR[:, :, W], XS[:, :, W - 1], OP.add)
            state["dl", it] = DL
            state["dr", it] = DR

        def stage_sample(it):
            """A/Av maps + horizontal interps + vertical combine -> OS."""
            XS = state["xs", it]
            XSf = _f(XS[:])
            SX, SY = state["sx", it], state["sy", it]
            B2, BV2 = state["b2", it], state["bv2", it]
            DLf, DRf = _f(state["dl", it]), _f(state["dr", it])
            # A = relu(sx-.5) = (sx max .5) - .5, in place over SX (DVE 4x)
            nc.vector.tensor_scalar(_f(SX), _f(SX), 0.5, 0.5, OP.max,
                                    OP.subtract)
            nc.vector.tensor_scalar(_f(SY), _f(SY), 0.5, 0.5, OP.max,
                                    OP.subtract)
            A, AV = SX, SY
            Af, B2f = _f(A), _f(B2)

            Lh = nh * WP
            # dr=-1's A*DL runs on gpsimd (own buffer, issued first so the
            # gpsimd engine starts as soon as A is ready)
            T1M = hpool.tile([128, nh, WP], BF16, tag="htg")
            nc.gpsimd.tensor_tensor(_f(T1M), Af, DLf[:, 0:Lh], OP.mult)
            Hs = []
            # dr=-1 last: its T1 comes from gpsimd and is ready mid-window
            for dr in (0, 1, -1):
                off = (1 + dr) * WP
                if dr == -1:
                    T1 = T1M
                else:
                    T1 = hpool.tile([128, nh, WP], BF16, tag="ht1", name="t1")
                    nc.vector.tensor_tensor(
                        _f(T1), Af, DLf[:, off:off + Lh], OP.mult)
                T2 = hpool.tile([128, nh, WP], BF16, tag="ht2")
                nc.vector.tensor_tensor(
                    _f(T2), B2f, DRf[:, off:off + Lh], OP.mult)
                Hd = hpool.tile([128, nh, WP], BF16, tag=f"h{dr}", name=f"h{dr}")
                nc.vector.tensor_tensor(
                    _f(Hd), _f(T1), XSf[:, off:off + Lh], OP.add)
                nc.vector.tensor_tensor(_f(Hd), _f(Hd), _f(T2), OP.add)
                Hs.append(Hd)
            H0, Hp, Hm = Hs

            # ---- vertical combine: OS = H0 + Av*(Hm-H0) + Bv2*(Hp-H0) ----
            nc.vector.tensor_tensor(_f(Hm), _f(Hm), _f(H0), OP.subtract)
            nc.vector.tensor_tensor(_f(Hp), _f(Hp), _f(H0), OP.subtract)
            T3 = hpool.tile([128, nh, WP], BF16, tag="ht1")
            nc.vector.tensor_tensor(_f(T3), _f(AV), _f(Hm), OP.mult)
            T4 = hpool.tile([128, nh, WP], BF16, tag="ht2")
            nc.vector.tensor_tensor(_f(T4), _f(BV2), _f(Hp), OP.mult)
            # reflect fixups at image rows 0 / H-1: swap the dead term
            if it == 0:
                nc.vector.tensor_tensor(
                    _f(T3[0:32, 1:2, :]), _f(AV[0:32, 1:2, :]),
                    _f(Hp[0:32, 1:2, :]), OP.mult)
            if it == nslab - 1:
                nc.vector.tensor_tensor(
                    _f(T4[96:128, nr:nr + 1, :]), _f(BV2[96:128, nr:nr + 1, :]),
                    _f(Hm[96:128, nr:nr + 1, :]), OP.mult)
            OS = ospool.tile([128, nh, WP], BF16, tag="os")
            nc.vector.tensor_tensor(_f(OS), _f(H0), _f(T3), OP.add)
            nc.vector.tensor_tensor(_f(OS), _f(OS), _f(T4), OP.add)
            # sampled outside the image is 0 for the final conv zero-padding
            nc.vector.memset(OS[:, :, 0:WP:W + 1], 0.0)
            if it == 0:
                nc.vector.memset(_f(OS[0:32, 0:1, :]), 0.0)
            if it == nslab - 1:
                nc.vector.memset(_f(OS[96:128, nr + 1:nr + 2, :]), 0.0)
            state["os", it] = OS

        def stage_outconv(it):
            """conv_dcn + bias + store (4-row output chunks)."""
            OSf = _f(state["os", it])
            r0 = it * nr
            OROWS = 4
            for oc_i in range(nr // OROWS):
                OC_t = ocpool.tile([128, OROWS, WP], F32, tag="oc")
                for oj in range(OROWS):
                    oi = oc_i * OROWS + oj
                    ot = opool.tile([128, 512], F32, tag="ot")
                    for t in range(9):
                        kh, kw = t // 3, t % 3
                        base = (oi + kh) * WP + kw
                        for g in range(4):
                            nc.tensor.matmul(
                                ot[32 * g:32 * g + 32, 1:W + 1],
                                lhsT=WDCN[32 * g:32 * g + 32, t, :],
                                rhs=OSf[32 * g:32 * g + 32, base:base + W],
                                start=(t == 0), stop=(t == 8),
                                tile_position=(32 * g, 32 * g),
                                skip_group_check=True)
                    nc.scalar.activation(
                        out=OC_t[:, oj, 1:W + 1], in_=ot[:, 1:W + 1],
                        func=AF.Identity, bias=BDCN[:], scale=1.0)
                for g in range(4):
                    rr = cfg.QH * g + r0 + oc_i * OROWS
                    nc.sync.dma_start(
                        out=y_out[:, rr:rr + OROWS, :],
                        in_=OC_t[32 * g:32 * g + 32, :, 1:W + 1])

        # ---- software-pipelined slab loop ----
        # emission order per iteration: offset(i) | sample(i-1), outconv(i-1)
        # | diff(i), load(i+1).  diff/load go last so their bufs=1/2 slots
        # rotate only after the previous slab's reads were emitted.
        load_slab(0)
        for it in range(nslab + 1):
            if it < nslab:
                stage_offset(it)
            if it > 0:
                stage_sample(it - 1)
                stage_outconv(it - 1)
            if it < nslab:
                stage_diff(it)
            if it + 1 < nslab:
                load_slab(it + 1)
    if finalize:
        nc.finalize()
    return nc


def prep_weights(w_off, b_off, w_dcn, b_dcn):
    """Host-side packing of conv weights into lhsT tiles, replicated x4."""
    perm = np.concatenate([np.arange(0, 2 * C, 2), np.arange(1, 2 * C, 2)])
    # WOFF[32g+ci, kh*3+kw, m] = w_off[perm[m], ci, kh, kw]
    wo = w_off[perm].astype(np.float32)            # [64, C, 3, 3]
    wo = wo.transpose(1, 2, 3, 0).reshape(C, 9, OC2)   # [ci, tap, m]
    woff = np.tile(wo, (4, 1, 1)).reshape(128, 9 * OC2)
    wd = w_dcn.astype(np.float32).transpose(1, 2, 3, 0).reshape(C, 9, C)
    wdcn = np.tile(wd, (4, 1, 1)).reshape(128, 9 * C)
    boff = np.tile(b_off[perm].astype(np.float32), 2).reshape(128, 1)
    bdcn = np.tile(b_dcn.astype(np.float32), 4).reshape(128, 1)
    return {
        "woff": woff.astype(ml_dtypes.bfloat16),
        "wdcn": wdcn.astype(ml_dtypes.bfloat16),
        "boff": boff.astype(np.float32),
        "bdcn": bdcn.astype(np.float32),
    }


_NC_CACHE = {}


def _get_nc(cfg_key):
    if cfg_key not in _NC_CACHE:
        _NC_CACHE[cfg_key] = build_nc(Cfg(H=cfg_key[0], nr=cfg_key[1]))
    return _NC_CACHE[cfg_key]


def _run(x, w_off, b_off, w_dcn, b_dcn, **spmd_kwargs):
    from concourse.bass_utils import run_bass_kernel_spmd

    B = x.shape[0]
    H = x.shape[2]
    assert x.shape == (B, C, H, H) and B == N_CORES
    nc = _get_nc((H, 8))
    w = prep_weights(np.asarray(w_off), np.asarray(b_off),
                     np.asarray(w_dcn), np.asarray(b_dcn))
    in_maps = []
    for b in range(B):
        m = dict(w)
        xb = np.asarray(x[b]).astype(ml_dtypes.bfloat16)
        m["x"] = np.pad(xb, ((0, 0), (2, 2), (0, 0)))
        in_maps.append(m)
    return run_bass_kernel_spmd(nc, in_maps, list(range(N_CORES)), **spmd_kwargs)


def kernel(x, w_off, b_off, w_dcn, b_dcn):
    res = _run(x, w_off, b_off, w_dcn, b_dcn)
    out = np.stack([res.results[i]["y"] for i in range(N_CORES)], axis=0)
    return out.astype(np.float32)


# revision 7
# speedup vs baseline: 1.9190x; 1.4376x over previous
"""DCN block kernel for Trainium2 (8 NeuronCores, data-parallel over batch).

Math (per batch b, plane c):
  z   = conv3x3(x, w_off) + b_off                  (64 offset logits)
  s   = sigmoid(z); d = s - 0.5 in (-.5,.5)        (pixel displacement)
  sample at (r - dy, c - dx) bilinear w/ reflect   (|d| < .5 => 3x3 support)
  y   = conv3x3(sampled, w_dcn) + b_dcn

Gather-free sampling via relu-split weights (all DVE ops run in 2x/4x mode):
  A  = relu(sx-.5)  B = relu(.5-sx)   (horizontal taps)
  H_dr = x_dr + A*DL_dr + B*DR_dr     DL = x(c-1)-x(c), DR = x(c+1)-x(c)
  OS = H0 + Av*(Hm-H0) + Bv*(Hp-H0)   (vertical combine)
Reflect at borders: column fixups on DL/DR; row fixups swap the dead Hm/Hp
term at the global top/bottom rows.

Layout: 4 image row-quarters stacked on partition groups [4 x 32ch]; convs
run as 4 concurrent row-tiled matmul streams (tile_position), 9 taps
accumulating in PSUM.

Two schedule tricks:
 - the offset stage for slab i+1 is issued BEFORE the sampling stage of
   slab i, so PE/scalar work overlaps the vector pipeline;
 - slabs are incremental: the 2 sampled halo rows each slab shares with its
   predecessor are copied from the previous OS tile instead of recomputed,
   so conv_off/sigmoid/sampling all run on nr rows, not nr+2.
"""

from contextlib import ExitStack

import ml_dtypes
import numpy as np

import concourse.bacc as bacc
import concourse.bass as bass
import concourse.mybir as mybir
import concourse.tile as tile

BF16 = mybir.dt.bfloat16
F32 = mybir.dt.float32
AF = mybir.ActivationFunctionType
OP = mybir.AluOpType

N_CORES = 8
C = 32          # input/output channels per plane set
OC2 = 64        # offset logits (2 per plane)


class Cfg:
    def __init__(self, H=384, nr=8):
        self.H = H
        self.W = H
        self.WP = self.W + 2          # padded row: [pad, 0..W-1, pad]
        self.QH = H // 4              # rows per quarter
        assert self.QH % nr == 0
        self.nr = nr                  # output rows per quarter per slab
        self.nslab = self.QH // nr


def _f(ap):
    """Flatten free dims of a 3d tile AP to [P, fd]."""
    return ap.rearrange("p a b -> p (a b)")


def build_nc(cfg: Cfg, finalize=True):
    nc = bacc.Bacc()
    H, W, WP, nr = cfg.H, cfg.W, cfg.WP, cfg.nr
    nh = nr + 2   # OS rows:  [r0-1, r0+nr+1)
    nx = nr + 4   # x rows:   [r0-2, r0+nr+2)
    nslab = cfg.nslab

    x_in = nc.declare_dram_parameter("x", [C, H + 4, W], BF16, isOutput=False)
    woff_in = nc.declare_dram_parameter("woff", [128, 9 * OC2], BF16, isOutput=False)
    wdcn_in = nc.declare_dram_parameter("wdcn", [128, 9 * C], BF16, isOutput=False)
    boff_in = nc.declare_dram_parameter("boff", [128, 1], F32, isOutput=False)
    bdcn_in = nc.declare_dram_parameter("bdcn", [128, 1], F32, isOutput=False)
    y_out = nc.declare_dram_parameter("y", [C, H, W], F32, isOutput=True)

    with tile.TileContext(nc) as tc, ExitStack() as ctx:
        consts = ctx.enter_context(tc.tile_pool(name="consts", bufs=1))
        xpool = ctx.enter_context(tc.tile_pool(name="xp", bufs=2))
        spool = ctx.enter_context(tc.tile_pool(name="sp", bufs=1))
        sxpool = ctx.enter_context(tc.tile_pool(name="sxp", bufs=2))
        bpool = ctx.enter_context(tc.tile_pool(name="bp", bufs=2))
        dpool = ctx.enter_context(tc.tile_pool(name="dp", bufs=1))
        hpool = ctx.enter_context(tc.tile_pool(name="hp", bufs=1))
        ospool = ctx.enter_context(tc.tile_pool(name="osp", bufs=2))
        ocpool = ctx.enter_context(tc.tile_pool(name="ocp", bufs=2))
        zpool = ctx.enter_context(tc.tile_pool(name="zp", bufs=2, space="PSUM"))
        opool = ctx.enter_context(tc.tile_pool(name="op", bufs=2, space="PSUM"))

        WOFF = consts.tile([128, 9, OC2], BF16)
        nc.sync.dma_start(out=_f(WOFF), in_=woff_in[:])
        WDCN = consts.tile([128, 9, C], BF16)
        nc.sync.dma_start(out=_f(WDCN), in_=wdcn_in[:])
        BOFF = consts.tile([128, 1], F32)
        nc.sync.dma_start(out=BOFF[:], in_=boff_in[:])
        BDCN = consts.tile([128, 1], F32)
        nc.sync.dma_start(out=BDCN[:], in_=bdcn_in[:])
        PHALF = consts.tile([128, 1], F32)
        nc.vector.memset(PHALF[:], 0.5)

        # persistent x slabs: pre-zero once so pad cols stay zero forever
        XS_list = []
        for sl in range(2):
            XSz = xpool.tile([128, nx, WP], BF16, tag="xs", name=f"xsz{sl}")
            nc.vector.memset(_f(XSz), 0.0)
            XS_list.append(XSz)

        state = {}   # per-slab live tiles, keyed (name, slab)

        def zlo_of(it):
            # slab 0 computes its top halo rows; later slabs copy them from
            # the previous slab's OS tile and only compute OS rows
            # [2, nr+2) == image rows r0+1 .. r0+nr.
            return 0 if it == 0 else 2

        def load_slab(it):
            if it < 2:
                XS = XS_list[it % 2]
            else:
                XS = xpool.tile([128, nx, WP], BF16, tag="xs", name="xs")
            state["xs", it] = XS
            flo = 0 if it == 0 else 2   # steady slabs only read frame rows 2+
            for g in range(4):
                i0 = cfg.QH * g + it * nr   # row in the padded-x frame
                nc.sync.dma_start(
                    out=XS[32 * g:32 * g + 32, flo:nx, 1:W + 1],
                    in_=x_in[:, i0 + flo:i0 + nx, :])

        def stage_offset(it):
            """conv_off + sigmoid + fold + B2/Bv2 acts for slab it."""
            XS = state["xs", it]
            XSf = _f(XS[:])
            zlo = zlo_of(it)
            nz = nr + 2 - zlo
            # conv_off: pairs (q0,q1)->zts[0], (q2,q3)->zts[1]
            SP = [spool.tile([128, nz, WP], BF16, tag=f"sp{p}", name=f"sp{p}")
                  for p in range(2)]
            for j in range(nz):
                zts = [zpool.tile([128, 512], F32, tag=f"z{p}", name=f"z{p}")
                       for p in range(2)]
                for t in range(9):
                    kh, kw = t // 3, t % 3
                    base = (zlo + j + kh) * WP + kw
                    for p in range(2):
                        for gq in range(2):
                            g = 2 * p + gq
                            nc.tensor.matmul(
                                zts[p][64 * gq:64 * gq + 64, 1:W + 1],
                                lhsT=WOFF[32 * g:32 * g + 32, t, :],
                                rhs=XSf[32 * g:32 * g + 32, base:base + W],
                                start=(t == 0), stop=(t == 8),
                                tile_position=(32 * g, 64 * gq),
                                skip_group_check=True)
                for p in range(2):
                    nc.scalar.activation(
                        out=SP[p][:, j, 1:W + 1], in_=zts[p][:, 1:W + 1],
                        func=AF.Sigmoid, bias=BOFF[:], scale=1.0)
            # fold s into quarter-stacked SX/SY (8 sbuf->sbuf DMAs)
            SX = sxpool.tile([128, nz, WP], BF16, tag="sx")
            SY = sxpool.tile([128, nz, WP], BF16, tag="sy")
            for g in range(4):
                p, gq = g // 2, g % 2
                nc.sync.dma_start(
                    out=_f(SX[32 * g:32 * g + 32]),
                    in_=_f(SP[p][64 * gq:64 * gq + 32]))
                nc.sync.dma_start(
                    out=_f(SY[32 * g:32 * g + 32]),
                    in_=_f(SP[p][64 * gq + 32:64 * gq + 64]))
            # B maps (scalar engine): B2 = relu(.5-sx), Bv2 = relu(.5-sy)
            B2 = bpool.tile([128, nz, WP], BF16, tag="b2")
            nc.scalar.activation(out=_f(B2), in_=_f(SX), func=AF.Relu,
                                 bias=PHALF[:], scale=-1.0)
            BV2 = bpool.tile([128, nz, WP], BF16, tag="bv2")
            nc.scalar.activation(out=_f(BV2), in_=_f(SY), func=AF.Relu,
                                 bias=PHALF[:], scale=-1.0)
            state["sx", it] = SX
            state["sy", it] = SY
            state["b2", it] = B2
            state["bv2", it] = BV2

        def stage_sample(it):
            """DL/DR + A/Av maps + horizontal interps + vertical -> OS."""
            XS = state["xs", it]
            XSf = _f(XS[:])
            zlo = zlo_of(it)
            nz = nr + 2 - zlo
            Lz = nz * WP
            # difference images (no deps beyond XS -> issued first so the
            # vector engine has work while the offset chain finishes)
            L = nx * WP
            DL = dpool.tile([128, nx, WP], BF16, tag="dl")
            DLf = _f(DL)
            nc.vector.tensor_tensor(
                DLf[:, 1:L], XSf[:, 0:L - 1], XSf[:, 1:L], OP.subtract)
            DR = dpool.tile([128, nx, WP], BF16, tag="dr")
            DRf = _f(DR)
            nc.vector.tensor_tensor(
                DRf[:, 0:L - 1], XSf[:, 1:L], XSf[:, 0:L - 1], OP.subtract)
            # reflect fixups at image cols 0 / W-1 (padded cols 1 / W)
            nc.vector.tensor_tensor(
                DL[:, :, 1], DL[:, :, 1], XS[:, :, 2], OP.add)
            nc.vector.tensor_tensor(
                DR[:, :, W], DR[:, :, W], XS[:, :, W - 1], OP.add)
            # copy the 2 shared halo rows of OS from the previous slab
            OS = ospool.tile([128, nh, WP], BF16, tag="os")
            if it > 0:
                OSprev = state["os", it - 1]
                nc.vector.tensor_copy(
                    _f(OS[:, 0:2, :]), _f(OSprev[:, nr:nr + 2, :]))
                del state["os", it - 1]
            # A = relu(sx-.5) = (sx max .5) - .5, in place over SX (DVE 4x)
            SX, SY = state["sx", it], state["sy", it]
            B2, BV2 = state["b2", it], state["bv2", it]
            nc.vector.tensor_scalar(_f(SX), _f(SX), 0.5, 0.5, OP.max,
                                    OP.subtract)
            nc.vector.tensor_scalar(_f(SY), _f(SY), 0.5, 0.5, OP.max,
                                    OP.subtract)
            A, AV = SX, SY
            Af, B2f = _f(A), _f(B2)

            Hs = []
            for dr in (0, 1, -1):
                off = (zlo + 1 + dr) * WP
                T1 = hpool.tile([128, nz, WP], BF16, tag="ht1", name="t1")
                nc.vector.tensor_tensor(
                    _f(T1), Af, DLf[:, off:off + Lz], OP.mult)
                T2 = hpool.tile([128, nz, WP], BF16, tag="ht2", name="t2")
                nc.vector.tensor_tensor(
                    _f(T2), B2f, DRf[:, off:off + Lz], OP.mult)
                Hd = hpool.tile([128, nz, WP], BF16, tag=f"h{dr}", name=f"h{dr}")
                nc.vector.tensor_tensor(
                    _f(Hd), _f(T1), XSf[:, off:off + Lz], OP.add)
                nc.vector.tensor_tensor(_f(Hd), _f(Hd), _f(T2), OP.add)
                Hs.append(Hd)
            H0, Hp, Hm = Hs

            # ---- vertical combine: OS = H0 + Av*(Hm-H0) + Bv2*(Hp-H0) ----
            nc.vector.tensor_tensor(_f(Hm), _f(Hm), _f(H0), OP.subtract)
            nc.vector.tensor_tensor(_f(Hp), _f(Hp), _f(H0), OP.subtract)
            T3 = hpool.tile([128, nz, WP], BF16, tag="ht1", name="t3")
            nc.vector.tensor_tensor(_f(T3), _f(AV), _f(Hm), OP.mult)
            T4 = hpool.tile([128, nz, WP], BF16, tag="ht2", name="t4")
            nc.vector.tensor_tensor(_f(T4), _f(BV2), _f(Hp), OP.mult)
            # reflect fixups at image rows 0 / H-1: swap the dead term
            if it == 0:
                nc.vector.tensor_tensor(
                    _f(T3[0:32, 1:2, :]), _f(AV[0:32, 1:2, :]),
                    _f(Hp[0:32, 1:2, :]), OP.mult)
            if it == nslab - 1:
                jb = nr - zlo   # OS row nr == image row H-1 (quarter 3)
                nc.vector.tensor_tensor(
                    _f(T4[96:128, jb:jb + 1, :]), _f(BV2[96:128, jb:jb + 1, :]),
                    _f(Hm[96:128, jb:jb + 1, :]), OP.mult)
            OSc = _f(OS[:, zlo:nr + 2, :])
            nc.vector.tensor_tensor(OSc, _f(H0), _f(T3), OP.add)
            nc.vector.tensor_tensor(OSc, OSc, _f(T4), OP.add)
            # sampled outside the image is 0 for the final conv zero-padding
            nc.vector.memset(OS[:, :, 0:WP:W + 1], 0.0)
            if it == 0:
                nc.vector.memset(_f(OS[0:32, 0:1, :]), 0.0)
            if it == nslab - 1:
                nc.vector.memset(_f(OS[96:128, nr + 1:nr + 2, :]), 0.0)
            state["os", it] = OS

        def stage_outconv(it):
            """conv_dcn + bias + store (4-row output chunks)."""
            OSf = _f(state["os", it])
            r0 = it * nr
            OROWS = 4
            for oc_i in range(nr // OROWS):
                OC_t = ocpool.tile([128, OROWS, WP], F32, tag="oc")
                for oj in range(OROWS):
                    oi = oc_i * OROWS + oj
                    ot = opool.tile([128, 512], F32, tag="ot")
                    for t in range(9):
                        kh, kw = t // 3, t % 3
                        base = (oi + kh) * WP + kw
                        for g in range(4):
                            nc.tensor.matmul(
                                ot[32 * g:32 * g + 32, 1:W + 1],
                                lhsT=WDCN[32 * g:32 * g + 32, t, :],
                                rhs=OSf[32 * g:32 * g + 32, base:base + W],
                                start=(t == 0), stop=(t == 8),
                                tile_position=(32 * g, 32 * g),
                                skip_group_check=True)
                    nc.scalar.activation(
                        out=OC_t[:, oj, 1:W + 1], in_=ot[:, 1:W + 1],
                        func=AF.Identity, bias=BDCN[:], scale=1.0)
                for g in range(4):
                    rr = cfg.QH * g + r0 + oc_i * OROWS
                    nc.sync.dma_start(
                        out=y_out[:, rr:rr + OROWS, :],
                        in_=OC_t[32 * g:32 * g + 32, :, 1:W + 1])

        # ---- software-pipelined slab loop ----
        # emission order per iteration: offset(i) | sample(i-1), outconv(i-1)
        # | load(i+1), so the PE's conv_off(i) overlaps the vector work of
        # sample(i-1).
        load_slab(0)
        for it in range(nslab + 1):
            if it < nslab:
                stage_offset(it)
            if it > 0:
                stage_sample(it - 1)
                stage_outconv(it - 1)
            if it + 1 < nslab:
                load_slab(it + 1)
    if finalize:
        nc.finalize()
    return nc


def prep_weights(w_off, b_off, w_dcn, b_dcn):
    """Host-side packing of conv weights into lhsT tiles, replicated x4."""
    perm = np.concatenate([np.arange(0, 2 * C, 2), np.arange(1, 2 * C, 2)])
    # WOFF[32g+ci, kh*3+kw, m] = w_off[perm[m], ci, kh, kw]
    wo = w_off[perm].astype(np.float32)            # [64, C, 3, 3]
    wo = wo.transpose(1, 2, 3, 0).reshape(C, 9, OC2)   # [ci, tap, m]
    woff = np.tile(wo, (4, 1, 1)).reshape(128, 9 * OC2)
    wd = w_dcn.astype(np.float32).transpose(1, 2, 3, 0).reshape(C, 9, C)
    wdcn = np.tile(wd, (4, 1, 1)).reshape(128, 9 * C)
    boff = np.tile(b_off[perm].astype(np.float32), 2).reshape(128, 1)
    bdcn = np.tile(b_dcn.astype(np.float32), 4).reshape(128, 1)
    return {
        "woff": woff.astype(ml_dtypes.bfloat16),
        "wdcn": wdcn.astype(ml_dtypes.bfloat16),
        "boff": boff.astype(np.float32),
        "bdcn": bdcn.astype(np.float32),
    }


_NC_CACHE = {}


def _get_nc(cfg_key):
    if cfg_key not in _NC_CACHE:
        _NC_CACHE[cfg_key] = build_nc(Cfg(H=cfg_key[0], nr=cfg_key[1]))
    return _NC_CACHE[cfg_key]


def _run(x, w_off, b_off, w_dcn, b_dcn, **spmd_kwargs):
    from concourse.bass_utils import run_bass_kernel_spmd

    B = x.shape[0]
    H = x.shape[2]
    assert x.shape == (B, C, H, H) and B == N_CORES
    nc = _get_nc((H, 8))
    w = prep_weights(np.asarray(w_off), np.asarray(b_off),
                     np.asarray(w_dcn), np.asarray(b_dcn))
    in_maps = []
    for b in range(B):
        m = dict(w)
        xb = np.asarray(x[b]).astype(ml_dtypes.bfloat16)
        m["x"] = np.pad(xb, ((0, 0), (2, 2), (0, 0)))
        in_maps.append(m)
    return run_bass_kernel_spmd(nc, in_maps, list(range(N_CORES)), **spmd_kwargs)


def kernel(x, w_off, b_off, w_dcn, b_dcn):
    res = _run(x, w_off, b_off, w_dcn, b_dcn)
    out = np.stack([res.results[i]["y"] for i in range(N_CORES)], axis=0)
    return out.astype(np.float32)


# revision 10
# speedup vs baseline: 1.9674x; 1.0252x over previous
"""DCN block kernel for Trainium2 (8 NeuronCores, data-parallel over batch).

Math (per batch b, plane c):
  z   = conv3x3(x, w_off) + b_off                  (64 offset logits)
  s   = sigmoid(z); d = s - 0.5 in (-.5,.5)        (pixel displacement)
  sample at (r - dy, c - dx) bilinear w/ reflect   (|d| < .5 => 3x3 support)
  y   = conv3x3(sampled, w_dcn) + b_dcn

Gather-free sampling via relu-split weights (all DVE ops run in 2x/4x mode):
  A  = relu(sx-.5)  B = relu(.5-sx)   (horizontal taps)
  H_dr = x_dr + A*DL_dr + B*DR_dr     DL = x(c-1)-x(c), DR = x(c+1)-x(c)
  OS = H0 + Av*(Hm-H0) + Bv*(Hp-H0)   (vertical combine)
Reflect at borders: column fixups on DL/DR; row fixups swap the dead Hm/Hp
term at the global top/bottom rows.

Layout: 4 image row-quarters stacked on partition groups [4 x 32ch]; convs
run as 4 concurrent row-tiled matmul streams (tile_position), 9 taps
accumulating in PSUM.

Two schedule tricks:
 - the offset stage for slab i+1 is issued BEFORE the sampling stage of
   slab i, so PE/scalar work overlaps the vector pipeline;
 - slabs are incremental: the 2 sampled halo rows each slab shares with its
   predecessor are copied from the previous OS tile instead of recomputed,
   so conv_off/sigmoid/sampling all run on nr rows, not nr+2.
"""

from contextlib import ExitStack

import ml_dtypes
import numpy as np

import concourse.bacc as bacc
import concourse.bass as bass
import concourse.mybir as mybir
import concourse.tile as tile

BF16 = mybir.dt.bfloat16
F32 = mybir.dt.float32
AF = mybir.ActivationFunctionType
OP = mybir.AluOpType

N_CORES = 8
C = 32          # input/output channels per plane set
OC2 = 64        # offset logits (2 per plane)


class Cfg:
    def __init__(self, H=384, nr=8):
        self.H = H
        self.W = H
        self.WP = self.W + 2          # padded row: [pad, 0..W-1, pad]
        self.QH = H // 4              # rows per quarter
        assert self.QH % nr == 0
        self.nr = nr                  # output rows per quarter per slab
        self.nslab = self.QH // nr


def _f(ap):
    """Flatten free dims of a 3d tile AP to [P, fd]."""
    return ap.rearrange("p a b -> p (a b)")


def build_nc(cfg: Cfg, finalize=True):
    nc = bacc.Bacc()
    H, W, WP, nr = cfg.H, cfg.W, cfg.WP, cfg.nr
    nh = nr + 2   # OS rows:  [r0-1, r0+nr+1)
    nx = nr + 4   # x rows:   [r0-2, r0+nr+2)
    nslab = cfg.nslab

    x_in = nc.declare_dram_parameter("x", [C, H + 4, W], BF16, isOutput=False)
    woff_in = nc.declare_dram_parameter("woff", [128, 9 * OC2], BF16, isOutput=False)
    wdcn_in = nc.declare_dram_parameter("wdcn", [128, 9 * C], BF16, isOutput=False)
    boff_in = nc.declare_dram_parameter("boff", [128, 1], F32, isOutput=False)
    bdcn_in = nc.declare_dram_parameter("bdcn", [128, 1], F32, isOutput=False)
    y_out = nc.declare_dram_parameter("y", [C, H, W], F32, isOutput=True)

    with tile.TileContext(nc) as tc, ExitStack() as ctx:
        consts = ctx.enter_context(tc.tile_pool(name="consts", bufs=1))
        xpool = ctx.enter_context(tc.tile_pool(name="xp", bufs=2))
        spool = ctx.enter_context(tc.tile_pool(name="sp", bufs=1))
        sxpool = ctx.enter_context(tc.tile_pool(name="sxp", bufs=2))
        bpool = ctx.enter_context(tc.tile_pool(name="bp", bufs=2))
        dpool = ctx.enter_context(tc.tile_pool(name="dp", bufs=1))
        hpool = ctx.enter_context(tc.tile_pool(name="hp", bufs=1))
        ospool = ctx.enter_context(tc.tile_pool(name="osp", bufs=2))
        ocpool = ctx.enter_context(tc.tile_pool(name="ocp", bufs=2))
        zpool = ctx.enter_context(tc.tile_pool(name="zp", bufs=2, space="PSUM"))
        opool = ctx.enter_context(tc.tile_pool(name="op", bufs=2, space="PSUM"))

        WOFF = consts.tile([128, 9, OC2], BF16)
        nc.sync.dma_start(out=_f(WOFF), in_=woff_in[:])
        WDCN = consts.tile([128, 9, C], BF16)
        nc.sync.dma_start(out=_f(WDCN), in_=wdcn_in[:])
        BOFF = consts.tile([128, 1], F32)
        nc.sync.dma_start(out=BOFF[:], in_=boff_in[:])
        BDCN = consts.tile([128, 1], F32)
        nc.sync.dma_start(out=BDCN[:], in_=bdcn_in[:])
        PHALF = consts.tile([128, 1], F32)
        nc.vector.memset(PHALF[:], 0.5)

        # persistent x slabs: pre-zero once so pad cols stay zero forever
        XS_list = []
        for sl in range(2):
            XSz = xpool.tile([128, nx, WP], BF16, tag="xs", name=f"xsz{sl}")
            nc.vector.memset(_f(XSz), 0.0)
            XS_list.append(XSz)

        state = {}   # per-slab live tiles, keyed (name, slab)

        def zlo_of(it):
            # slab 0 computes its top halo rows; later slabs copy them from
            # the previous slab's OS tile and only compute OS rows
            # [2, nr+2) == image rows r0+1 .. r0+nr.
            return 0 if it == 0 else 2

        def load_slab(it):
            if it < 2:
                XS = XS_list[it % 2]
            else:
                XS = xpool.tile([128, nx, WP], BF16, tag="xs", name="xs")
            state["xs", it] = XS
            flo = 0 if it == 0 else 2   # steady slabs only read frame rows 2+
            for g in range(4):
                i0 = cfg.QH * g + it * nr   # row in the padded-x frame
                nc.sync.dma_start(
                    out=XS[32 * g:32 * g + 32, flo:nx, 1:W + 1],
                    in_=x_in[:, i0 + flo:i0 + nx, :])

        def stage_offset(it):
            """conv_off + sigmoid + fold + B2/Bv2 acts for slab it."""
            XS = state["xs", it]
            XSf = _f(XS[:])
            zlo = zlo_of(it)
            nz = nr + 2 - zlo
            # conv_off: pairs (q0,q1)->zts[0], (q2,q3)->zts[1]
            SP = spool.tile([128, 2, nz, WP], BF16, tag="sp", name="sp")
            for j in range(nz):
                zz = zpool.tile([128, 2, 512], F32, tag="zz", name="zz")
                for t in range(9):
                    kh, kw = t // 3, t % 3
                    base = (zlo + j + kh) * WP + kw
                    for p in range(2):
                        for gq in range(2):
                            g = 2 * p + gq
                            nc.tensor.matmul(
                                zz[64 * gq:64 * gq + 64, p, 1:W + 1],
                                lhsT=WOFF[32 * g:32 * g + 32, t, :],
                                rhs=XSf[32 * g:32 * g + 32, base:base + W],
                                start=(t == 0), stop=(t == 8),
                                tile_position=(32 * g, 64 * gq),
                                skip_group_check=True)
                nc.scalar.activation(
                    out=SP[:, :, j, 1:W + 1], in_=zz[:, :, 1:W + 1],
                    func=AF.Sigmoid, bias=BOFF[:], scale=1.0)
            # fold s into quarter-stacked SX/SY (8 sbuf->sbuf DMAs); SX first
            # so the sampling stage's A/B2 chain unblocks earliest
            SX = sxpool.tile([128, nz, WP], BF16, tag="sx")
            SY = sxpool.tile([128, nz, WP], BF16, tag="sy")
            for g in range(4):
                p, gq = g // 2, g % 2
                nc.sync.dma_start(
                    out=_f(SX[32 * g:32 * g + 32]),
                    in_=_f(SP[64 * gq:64 * gq + 32, p]))
            for g in range(4):
                p, gq = g // 2, g % 2
                nc.sync.dma_start(
                    out=_f(SY[32 * g:32 * g + 32]),
                    in_=_f(SP[64 * gq + 32:64 * gq + 64, p]))
            # B maps (scalar engine): B2 = relu(.5-sx), Bv2 = relu(.5-sy)
            B2 = bpool.tile([128, nz, WP], BF16, tag="b2")
            nc.scalar.activation(out=_f(B2), in_=_f(SX), func=AF.Relu,
                                 bias=PHALF[:], scale=-1.0)
            BV2 = bpool.tile([128, nz, WP], BF16, tag="bv2")
            nc.scalar.activation(out=_f(BV2), in_=_f(SY), func=AF.Relu,
                                 bias=PHALF[:], scale=-1.0)
            state["sx", it] = SX
            state["sy", it] = SY
            state["b2", it] = B2
            state["bv2", it] = BV2

        def stage_sample(it):
            """DL/DR + A/Av maps + horizontal interps + vertical -> OS."""
            XS = state["xs", it]
            XSf = _f(XS[:])
            zlo = zlo_of(it)
            nz = nr + 2 - zlo
            Lz = nz * WP
            # difference images (no deps beyond XS -> issued first so the
            # vector engine has work while the offset chain finishes)
            L = nx * WP
            DL = dpool.tile([128, nx, WP], BF16, tag="dl")
            DLf = _f(DL)
            nc.vector.tensor_tensor(
                DLf[:, 1:L], XSf[:, 0:L - 1], XSf[:, 1:L], OP.subtract)
            DR = dpool.tile([128, nx, WP], BF16, tag="dr")
            DRf = _f(DR)
            nc.vector.tensor_tensor(
                DRf[:, 0:L - 1], XSf[:, 1:L], XSf[:, 0:L - 1], OP.subtract)
            # reflect fixups at image cols 0 / W-1 (padded cols 1 / W)
            nc.vector.tensor_tensor(
                DL[:, :, 1], DL[:, :, 1], XS[:, :, 2], OP.add)
            nc.vector.tensor_tensor(
                DR[:, :, W], DR[:, :, W], XS[:, :, W - 1], OP.add)
            # copy the 2 shared halo rows of OS from the previous slab
            OS = ospool.tile([128, nh, WP], BF16, tag="os")
            if it > 0:
                OSprev = state["os", it - 1]
                nc.vector.tensor_copy(
                    _f(OS[:, 0:2, :]), _f(OSprev[:, nr:nr + 2, :]))
                del state["os", it - 1]
            # A = relu(sx-.5) = (sx max .5) - .5, in place over SX (DVE 4x)
            SX, SY = state["sx", it], state["sy", it]
            B2, BV2 = state["b2", it], state["bv2", it]
            nc.vector.tensor_scalar(_f(SX), _f(SX), 0.5, 0.5, OP.max,
                                    OP.subtract)
            A = SX
            Af, B2f = _f(A), _f(B2)

            Hs = []
            for dr in (0, 1, -1):
                off = (zlo + 1 + dr) * WP
                T1 = hpool.tile([128, nz, WP], BF16, tag="ht1", name="t1")
                nc.vector.tensor_tensor(
                    _f(T1), Af, DLf[:, off:off + Lz], OP.mult)
                T2 = hpool.tile([128, nz, WP], BF16, tag="ht2", name="t2")
                nc.vector.tensor_tensor(
                    _f(T2), B2f, DRf[:, off:off + Lz], OP.mult)
                Hd = hpool.tile([128, nz, WP], BF16, tag=f"h{dr}", name=f"h{dr}")
                nc.vector.tensor_tensor(
                    _f(Hd), _f(T1), XSf[:, off:off + Lz], OP.add)
                nc.vector.tensor_tensor(_f(Hd), _f(Hd), _f(T2), OP.add)
                Hs.append(Hd)
            H0, Hp, Hm = Hs

            # ---- vertical combine: OS = H0 + Av*(Hm-H0) + Bv2*(Hp-H0) ----
            # Av = relu(sy-.5), computed late: SY's fold lands after SX's
            nc.vector.tensor_scalar(_f(SY), _f(SY), 0.5, 0.5, OP.max,
                                    OP.subtract)
            AV = SY
            nc.vector.tensor_tensor(_f(Hm), _f(Hm), _f(H0), OP.subtract)
            nc.vector.tensor_tensor(_f(Hp), _f(Hp), _f(H0), OP.subtract)
            T3 = hpool.tile([128, nz, WP], BF16, tag="ht1", name="t3")
            nc.vector.tensor_tensor(_f(T3), _f(AV), _f(Hm), OP.mult)
            T4 = hpool.tile([128, nz, WP], BF16, tag="ht2", name="t4")
            nc.vector.tensor_tensor(_f(T4), _f(BV2), _f(Hp), OP.mult)
            # reflect fixups at image rows 0 / H-1: swap the dead term
            if it == 0:
                nc.vector.tensor_tensor(
                    _f(T3[0:32, 1:2, :]), _f(AV[0:32, 1:2, :]),
                    _f(Hp[0:32, 1:2, :]), OP.mult)
            if it == nslab - 1:
                jb = nr - zlo   # OS row nr == image row H-1 (quarter 3)
                nc.vector.tensor_tensor(
                    _f(T4[96:128, jb:jb + 1, :]), _f(BV2[96:128, jb:jb + 1, :]),
                    _f(Hm[96:128, jb:jb + 1, :]), OP.mult)
            OSc = _f(OS[:, zlo:nr + 2, :])
            nc.vector.tensor_tensor(OSc, _f(H0), _f(T3), OP.add)
            nc.vector.tensor_tensor(OSc, OSc, _f(T4), OP.add)
            # sampled outside the image is 0 for the final conv zero-padding
            nc.vector.memset(OS[:, :, 0:WP:W + 1], 0.0)
            if it == 0:
                nc.vector.memset(_f(OS[0:32, 0:1, :]), 0.0)
            if it == nslab - 1:
                nc.vector.memset(_f(OS[96:128, nr + 1:nr + 2, :]), 0.0)
            state["os", it] = OS

        def stage_outconv(it):
            """conv_dcn + bias + store (4-row output chunks)."""
            OSf = _f(state["os", it])
            r0 = it * nr
            OROWS = 4
            for oc_i in range(nr // OROWS):
                OC_t = ocpool.tile([128, OROWS, WP], F32, tag="oc")
                for oj in range(OROWS):
                    oi = oc_i * OROWS + oj
                    ot = opool.tile([128, 512], F32, tag="ot")
                    for t in range(9):
                        kh, kw = t // 3, t % 3
                        base = (oi + kh) * WP + kw
                        for g in range(4):
                            nc.tensor.matmul(
                                ot[32 * g:32 * g + 32, 1:W + 1],
                                lhsT=WDCN[32 * g:32 * g + 32, t, :],
                                rhs=OSf[32 * g:32 * g + 32, base:base + W],
                                start=(t == 0), stop=(t == 8),
                                tile_position=(32 * g, 32 * g),
                                skip_group_check=True)
                    nc.scalar.activation(
                        out=OC_t[:, oj, 1:W + 1], in_=ot[:, 1:W + 1],
                        func=AF.Identity, bias=BDCN[:], scale=1.0)
                for g in range(4):
                    rr = cfg.QH * g + r0 + oc_i * OROWS
                    nc.sync.dma_start(
                        out=y_out[:, rr:rr + OROWS, :],
                        in_=OC_t[32 * g:32 * g + 32, :, 1:W + 1])

        # ---- software-pipelined slab loop ----
        # emission order per iteration: offset(i) | sample(i-1), outconv(i-1)
        # | load(i+1), so the PE's conv_off(i) overlaps the vector work of
        # sample(i-1).
        load_slab(0)
        for it in range(nslab + 1):
            if it < nslab:
                stage_offset(it)
            if it > 0:
                stage_sample(it - 1)
                stage_outconv(it - 1)
            if it + 1 < nslab:
                load_slab(it + 1)
    if finalize:
        nc.finalize()
    return nc


def prep_weights(w_off, b_off, w_dcn, b_dcn):
    """Host-side packing of conv weights into lhsT tiles, replicated x4."""
    perm = np.concatenate([np.arange(0, 2 * C, 2), np.arange(1, 2 * C, 2)])
    # WOFF[32g+ci, kh*3+kw, m] = w_off[perm[m], ci, kh, kw]
    wo = w_off[perm].astype(np.float32)            # [64, C, 3, 3]
    wo = wo.transpose(1, 2, 3, 0).reshape(C, 9, OC2)   # [ci, tap, m]
    woff = np.tile(wo, (4, 1, 1)).reshape(128, 9 * OC2)
    wd = w_dcn.astype(np.float32).transpose(1, 2, 3, 0).reshape(C, 9, C)
    wdcn = np.tile(wd, (4, 1, 1)).reshape(128, 9 * C)
    boff = np.tile(b_off[perm].astype(np.float32), 2).reshape(128, 1)
    bdcn = np.tile(b_dcn.astype(np.float32), 4).reshape(128, 1)
    return {
        "woff": woff.astype(ml_dtypes.bfloat16),
        "wdcn": wdcn.astype(ml_dtypes.bfloat16),
        "boff": boff.astype(np.float32),
        "bdcn": bdcn.astype(np.float32),
    }


_NC_CACHE = {}


def _get_nc(cfg_key):
    if cfg_key not in _NC_CACHE:
        _NC_CACHE[cfg_key] = build_nc(Cfg(H=cfg_key[0], nr=cfg_key[1]))
    return _NC_CACHE[cfg_key]


def _run(x, w_off, b_off, w_dcn, b_dcn, **spmd_kwargs):
    from concourse.bass_utils import run_bass_kernel_spmd

    B = x.shape[0]
    H = x.shape[2]
    assert x.shape == (B, C, H, H) and B == N_CORES
    nc = _get_nc((H, 8))
    w = prep_weights(np.asarray(w_off), np.asarray(b_off),
                     np.asarray(w_dcn), np.asarray(b_dcn))
    in_maps = []
    for b in range(B):
        m = dict(w)
        xb = np.asarray(x[b]).astype(ml_dtypes.bfloat16)
        m["x"] = np.pad(xb, ((0, 0), (2, 2), (0, 0)))
        in_maps.append(m)
    return run_bass_kernel_spmd(nc, in_maps, list(range(N_CORES)), **spmd_kwargs)


def kernel(x, w_off, b_off, w_dcn, b_dcn):
    res = _run(x, w_off, b_off, w_dcn, b_dcn)
    out = np.stack([res.results[i]["y"] for i in range(N_CORES)], axis=0)
    return out.astype(np.float32)


# revision 11
# speedup vs baseline: 2.2265x; 1.1317x over previous
"""DCN block kernel for Trainium2 (8 NeuronCores, data-parallel over batch).

Math (per batch b, plane c):
  z   = conv3x3(x, w_off) + b_off                  (64 offset logits)
  s   = sigmoid(z); d = s - 0.5 in (-.5,.5)        (pixel displacement)
  sample at (r - dy, c - dx) bilinear w/ reflect   (|d| < .5 => 3x3 support)
  y   = conv3x3(sampled, w_dcn) + b_dcn

Gather-free sampling via relu-split weights (all DVE ops run in 2x/4x mode):
  A  = relu(sx-.5)  B = relu(.5-sx)   (horizontal taps)
  H_dr = x_dr + A*DL_dr + B*DR_dr     DL = x(c-1)-x(c), DR = x(c+1)-x(c)
  OS = H0 + Av*(Hm-H0) + Bv*(Hp-H0)   (vertical combine)
Reflect at borders: column fixups on DL/DR; row fixups swap the dead Hm/Hp
term at the global top/bottom rows.

Layout: 4 image row-quarters stacked on partition groups [4 x 32ch]; convs
run as 4 concurrent row-tiled matmul streams (tile_position), 9 taps
accumulating in PSUM.

Two schedule tricks:
 - the offset stage for slab i+1 is issued BEFORE the sampling stage of
   slab i, so PE/scalar work overlaps the vector pipeline;
 - slabs are incremental: the 2 sampled halo rows each slab shares with its
   predecessor are copied from the previous OS tile instead of recomputed,
   so conv_off/sigmoid/sampling all run on nr rows, not nr+2.
"""

from contextlib import ExitStack

import ml_dtypes
import numpy as np

import concourse.bacc as bacc
import concourse.bass as bass
import concourse.mybir as mybir
import concourse.tile as tile

BF16 = mybir.dt.bfloat16
F32 = mybir.dt.float32
AF = mybir.ActivationFunctionType
OP = mybir.AluOpType

N_CORES = 8
C = 32          # input/output channels per plane set
OC2 = 64        # offset logits (2 per plane)


class Cfg:
    def __init__(self, H=384, nr=8):
        self.H = H
        self.W = H
        self.WP = self.W + 2          # padded row: [pad, 0..W-1, pad]
        self.QH = H // 4              # rows per quarter
        assert self.QH % nr == 0
        self.nr = nr                  # output rows per quarter per slab
        self.nslab = self.QH // nr


def _f(ap):
    """Flatten free dims of a 3d tile AP to [P, fd]."""
    return ap.rearrange("p a b -> p (a b)")


def build_nc(cfg: Cfg, finalize=True):
    nc = bacc.Bacc()
    H, W, WP, nr = cfg.H, cfg.W, cfg.WP, cfg.nr
    nh = nr + 2   # OS rows:  [r0-1, r0+nr+1)
    nx = nr + 4   # x rows:   [r0-2, r0+nr+2)
    nslab = cfg.nslab

    x_in = nc.declare_dram_parameter("x", [C, H + 4, W], BF16, isOutput=False)
    woff_in = nc.declare_dram_parameter("woff", [128, 9 * OC2], BF16, isOutput=False)
    wdcn_in = nc.declare_dram_parameter("wdcn", [128, 9 * C], BF16, isOutput=False)
    boff_in = nc.declare_dram_parameter("boff", [128, 1], F32, isOutput=False)
    bdcn_in = nc.declare_dram_parameter("bdcn", [128, 1], F32, isOutput=False)
    y_out = nc.declare_dram_parameter("y", [C, H, W], F32, isOutput=True)

    with tile.TileContext(nc) as tc, ExitStack() as ctx:
        consts = ctx.enter_context(tc.tile_pool(name="consts", bufs=1))
        xpool = ctx.enter_context(tc.tile_pool(name="xp", bufs=2))
        spool = ctx.enter_context(tc.tile_pool(name="sp", bufs=1))
        sxpool = ctx.enter_context(tc.tile_pool(name="sxp", bufs=2))
        wpool = ctx.enter_context(tc.tile_pool(name="wp", bufs=1))
        dpool = ctx.enter_context(tc.tile_pool(name="dp", bufs=1))
        hpool = ctx.enter_context(tc.tile_pool(name="hp", bufs=1))
        ospool = ctx.enter_context(tc.tile_pool(name="osp", bufs=2))
        ocpool = ctx.enter_context(tc.tile_pool(name="ocp", bufs=2))
        zpool = ctx.enter_context(tc.tile_pool(name="zp", bufs=2, space="PSUM"))
        opool = ctx.enter_context(tc.tile_pool(name="op", bufs=2, space="PSUM"))

        WOFF = consts.tile([128, 9, OC2], BF16)
        nc.sync.dma_start(out=_f(WOFF), in_=woff_in[:])
        WDCN = consts.tile([128, 9, C], BF16)
        nc.sync.dma_start(out=_f(WDCN), in_=wdcn_in[:])
        BOFF = consts.tile([128, 1], F32)
        nc.sync.dma_start(out=BOFF[:], in_=boff_in[:])
        BDCN = consts.tile([128, 1], F32)
        nc.sync.dma_start(out=BDCN[:], in_=bdcn_in[:])
        PHALF = consts.tile([128, 1], F32)
        nc.vector.memset(PHALF[:], 0.5)

        # persistent x slabs: pre-zero once so pad cols stay zero forever
        XS_list = []
        for sl in range(2):
            XSz = xpool.tile([128, nx, WP], BF16, tag="xs", name=f"xsz{sl}")
            nc.vector.memset(_f(XSz), 0.0)
            XS_list.append(XSz)

        state = {}   # per-slab live tiles, keyed (name, slab)

        def zlo_of(it):
            # slab 0 computes its top halo rows; later slabs copy them from
            # the previous slab's OS tile and only compute OS rows
            # [2, nr+2) == image rows r0+1 .. r0+nr.
            return 0 if it == 0 else 2

        def load_slab(it):
            if it < 2:
                XS = XS_list[it % 2]
            else:
                XS = xpool.tile([128, nx, WP], BF16, tag="xs", name="xs")
            state["xs", it] = XS
            flo = 0 if it == 0 else 2   # steady slabs only read frame rows 2+
            for g in range(4):
                i0 = cfg.QH * g + it * nr   # row in the padded-x frame
                nc.sync.dma_start(
                    out=XS[32 * g:32 * g + 32, flo:nx, 1:W + 1],
                    in_=x_in[:, i0 + flo:i0 + nx, :])

        def stage_offset(it):
            """conv_off + sigmoid + fold + B2/Bv2 acts for slab it."""
            XS = state["xs", it]
            XSf = _f(XS[:])
            zlo = zlo_of(it)
            nz = nr + 2 - zlo
            # conv_off: pairs (q0,q1)->zts[0], (q2,q3)->zts[1]
            SP = spool.tile([128, 2, nz, WP], BF16, tag="sp", name="sp")
            for j in range(nz):
                zz = zpool.tile([128, 2, 512], F32, tag="zz", name="zz")
                for t in range(9):
                    kh, kw = t // 3, t % 3
                    base = (zlo + j + kh) * WP + kw
                    for p in range(2):
                        for gq in range(2):
                            g = 2 * p + gq
                            nc.tensor.matmul(
                                zz[64 * gq:64 * gq + 64, p, 1:W + 1],
                                lhsT=WOFF[32 * g:32 * g + 32, t, :],
                                rhs=XSf[32 * g:32 * g + 32, base:base + W],
                                start=(t == 0), stop=(t == 8),
                                tile_position=(32 * g, 64 * gq),
                                skip_group_check=True)
                nc.scalar.activation(
                    out=SP[:, :, j, 1:W + 1], in_=zz[:, :, 1:W + 1],
                    func=AF.Sigmoid, bias=BOFF[:], scale=1.0)
            # fold s into quarter-stacked SX/SY (8 sbuf->sbuf DMAs); SX first
            # so the sampling stage's A/B2 chain unblocks earliest
            SX = sxpool.tile([128, nz, WP], BF16, tag="sx")
            SY = sxpool.tile([128, nz, WP], BF16, tag="sy")
            for g in range(4):
                p, gq = g // 2, g % 2
                nc.sync.dma_start(
                    out=_f(SX[32 * g:32 * g + 32]),
                    in_=_f(SP[64 * gq:64 * gq + 32, p]))
            for g in range(4):
                p, gq = g // 2, g % 2
                nc.sync.dma_start(
                    out=_f(SY[32 * g:32 * g + 32]),
                    in_=_f(SP[64 * gq + 32:64 * gq + 64, p]))
            state["sx", it] = SX
            state["sy", it] = SY

        def stage_sample(it):
            """DL/DR + A/Av maps + horizontal interps + vertical -> OS."""
            XS = state["xs", it]
            XSf = _f(XS[:])
            zlo = zlo_of(it)
            nz = nr + 2 - zlo
            Lz = nz * WP
            # difference images (no deps beyond XS -> issued first so the
            # vector engine has work while the offset chain finishes)
            L = nx * WP
            DL = dpool.tile([128, nx, WP], BF16, tag="dl")
            DLf = _f(DL)
            nc.vector.tensor_tensor(
                DLf[:, 1:L], XSf[:, 0:L - 1], XSf[:, 1:L], OP.subtract)
            DR = dpool.tile([128, nx, WP], BF16, tag="dr")
            DRf = _f(DR)
            # DRn = x - x(c+1): sign-flipped so its weight can be the
            # vector-computable B2n = min(sx,.5)-.5 = -relu(.5-sx)
            nc.vector.tensor_tensor(
                DRf[:, 0:L - 1], XSf[:, 0:L - 1], XSf[:, 1:L], OP.subtract)
            # reflect fixups at image cols 0 / W-1 (padded cols 1 / W)
            nc.vector.tensor_tensor(
                DL[:, :, 1], DL[:, :, 1], XS[:, :, 2], OP.add)
            nc.vector.tensor_tensor(
                DR[:, :, W], DR[:, :, W], XS[:, :, W - 1], OP.subtract)
            # copy the 2 shared halo rows of OS from the previous slab
            OS = ospool.tile([128, nh, WP], BF16, tag="os")
            if it > 0:
                OSprev = state["os", it - 1]
                nc.vector.tensor_copy(
                    _f(OS[:, 0:2, :]), _f(OSprev[:, nr:nr + 2, :]))
                del state["os", it - 1]
            # weight maps, all on DVE (4x TS):
            #   B2n = min(sx,.5)-.5 = -relu(.5-sx)   (pairs with DRn)
            #   A   = max(sx,.5)-.5 =  relu(sx-.5)   (in place over SX,
            #         emitted after B2n so the read precedes the overwrite)
            SX, SY = state["sx", it], state["sy", it]
            B2 = wpool.tile([128, nz, WP], BF16, tag="b2n")
            nc.vector.tensor_scalar(_f(B2), _f(SX), 0.5, 0.5, OP.min,
                                    OP.subtract)
            nc.vector.tensor_scalar(_f(SX), _f(SX), 0.5, 0.5, OP.max,
                                    OP.subtract)
            A = SX
            Af, B2f = _f(A), _f(B2)

            Hs = []
            for dr in (0, 1, -1):
                off = (zlo + 1 + dr) * WP
                T1 = hpool.tile([128, nz, WP], BF16, tag="ht1", name="t1")
                nc.vector.tensor_tensor(
                    _f(T1), Af, DLf[:, off:off + Lz], OP.mult)
                T2 = hpool.tile([128, nz, WP], BF16, tag="ht2", name="t2")
                nc.vector.tensor_tensor(
                    _f(T2), B2f, DRf[:, off:off + Lz], OP.mult)
                Hd = hpool.tile([128, nz, WP], BF16, tag=f"h{dr}", name=f"h{dr}")
                nc.vector.tensor_tensor(
                    _f(Hd), _f(T1), XSf[:, off:off + Lz], OP.add)
                nc.vector.tensor_tensor(_f(Hd), _f(Hd), _f(T2), OP.add)
                Hs.append(Hd)
            H0, Hp, Hm = Hs

            # ---- vertical combine: OS = H0 + Av*(Hm-H0) + Bv2n*(H0-Hp) ----
            # Bv2n = min(sy,.5)-.5 (reads SY), then Av = relu(sy-.5) in place
            BV2 = wpool.tile([128, nz, WP], BF16, tag="bv2n")
            nc.vector.tensor_scalar(_f(BV2), _f(SY), 0.5, 0.5, OP.min,
                                    OP.subtract)
            nc.vector.tensor_scalar(_f(SY), _f(SY), 0.5, 0.5, OP.max,
                                    OP.subtract)
            AV = SY
            nc.vector.tensor_tensor(_f(Hm), _f(Hm), _f(H0), OP.subtract)
            nc.vector.tensor_tensor(_f(Hp), _f(H0), _f(Hp), OP.subtract)
            T3 = hpool.tile([128, nz, WP], BF16, tag="ht1", name="t3")
            nc.vector.tensor_tensor(_f(T3), _f(AV), _f(Hm), OP.mult)
            T4 = hpool.tile([128, nz, WP], BF16, tag="ht2", name="t4")
            nc.vector.tensor_tensor(_f(T4), _f(BV2), _f(Hp), OP.mult)
            # reflect fixups at image rows 0 / H-1: swap the dead term
            # (the swapped products need a sign flip in this formulation)
            if it == 0:
                nc.vector.tensor_tensor(
                    _f(T3[0:32, 1:2, :]), _f(AV[0:32, 1:2, :]),
                    _f(Hp[0:32, 1:2, :]), OP.mult)
                nc.vector.tensor_scalar_mul(
                    _f(T3[0:32, 1:2, :]), _f(T3[0:32, 1:2, :]), -1.0)
            if it == nslab - 1:
                jb = nr - zlo   # OS row nr == image row H-1 (quarter 3)
                nc.vector.tensor_tensor(
                    _f(T4[96:128, jb:jb + 1, :]), _f(BV2[96:128, jb:jb + 1, :]),
                    _f(Hm[96:128, jb:jb + 1, :]), OP.mult)
                nc.vector.tensor_scalar_mul(
                    _f(T4[96:128, jb:jb + 1, :]), _f(T4[96:128, jb:jb + 1, :]),
                    -1.0)
            OSc = _f(OS[:, zlo:nr + 2, :])
            nc.vector.tensor_tensor(OSc, _f(H0), _f(T3), OP.add)
            nc.vector.tensor_tensor(OSc, OSc, _f(T4), OP.add)
            # sampled outside the image is 0 for the final conv zero-padding
            nc.vector.memset(OS[:, :, 0:WP:W + 1], 0.0)
            if it == 0:
                nc.vector.memset(_f(OS[0:32, 0:1, :]), 0.0)
            if it == nslab - 1:
                nc.vector.memset(_f(OS[96:128, nr + 1:nr + 2, :]), 0.0)
            state["os", it] = OS

        def stage_outconv(it):
            """conv_dcn + bias + store (4-row output chunks)."""
            OSf = _f(state["os", it])
            r0 = it * nr
            OROWS = 4
            for oc_i in range(nr // OROWS):
                OC_t = ocpool.tile([128, OROWS, WP], F32, tag="oc")
                for oj in range(OROWS):
                    oi = oc_i * OROWS + oj
                    ot = opool.tile([128, 512], F32, tag="ot")
                    for t in range(9):
                        kh, kw = t // 3, t % 3
                        base = (oi + kh) * WP + kw
                        for g in range(4):
                            nc.tensor.matmul(
                                ot[32 * g:32 * g + 32, 1:W + 1],
                                lhsT=WDCN[32 * g:32 * g + 32, t, :],
                                rhs=OSf[32 * g:32 * g + 32, base:base + W],
                                start=(t == 0), stop=(t == 8),
                                tile_position=(32 * g, 32 * g),
                                skip_group_check=True)
                    nc.scalar.activation(
                        out=OC_t[:, oj, 1:W + 1], in_=ot[:, 1:W + 1],
                        func=AF.Identity, bias=BDCN[:], scale=1.0)
                for g in range(4):
                    rr = cfg.QH * g + r0 + oc_i * OROWS
                    nc.sync.dma_start(
                        out=y_out[:, rr:rr + OROWS, :],
                        in_=OC_t[32 * g:32 * g + 32, :, 1:W + 1])

        # ---- software-pipelined slab loop ----
        # emission order per iteration: offset(i) | sample(i-1), outconv(i-1)
        # | load(i+1), so the PE's conv_off(i) overlaps the vector work of
        # sample(i-1).
        load_slab(0)
        for it in range(nslab + 1):
            if it < nslab:
                stage_offset(it)
            if it > 0:
                stage_sample(it - 1)
                stage_outconv(it - 1)
            if it + 1 < nslab:
                load_slab(it + 1)
    if finalize:
        nc.finalize()
    return nc


def prep_weights(w_off, b_off, w_dcn, b_dcn):
    """Host-side packing of conv weights into lhsT tiles, replicated x4."""
    perm = np.concatenate([np.arange(0, 2 * C, 2), np.arange(1, 2 * C, 2)])
    # WOFF[32g+ci, kh*3+kw, m] = w_off[perm[m], ci, kh, kw]
    wo = w_off[perm].astype(np.float32)            # [64, C, 3, 3]
    wo = wo.transpose(1, 2, 3, 0).reshape(C, 9, OC2)   # [ci, tap, m]
    woff = np.tile(wo, (4, 1, 1)).reshape(128, 9 * OC2)
    wd = w_dcn.astype(np.float32).transpose(1, 2, 3, 0).reshape(C, 9, C)
    wdcn = np.tile(wd, (4, 1, 1)).reshape(128, 9 * C)
    boff = np.tile(b_off[perm].astype(np.float32), 2).reshape(128, 1)
    bdcn = np.tile(b_dcn.astype(np.float32), 4).reshape(128, 1)
    return {
        "woff": woff.astype(ml_dtypes.bfloat16),
        "wdcn": wdcn.astype(ml_dtypes.bfloat16),
        "boff": boff.astype(np.float32),
        "bdcn": bdcn.astype(np.float32),
    }


_NC_CACHE = {}


def _get_nc(cfg_key):
    if cfg_key not in _NC_CACHE:
        _NC_CACHE[cfg_key] = build_nc(Cfg(H=cfg_key[0], nr=cfg_key[1]))
    return _NC_CACHE[cfg_key]


def _run(x, w_off, b_off, w_dcn, b_dcn, **spmd_kwargs):
    from concourse.bass_utils import run_bass_kernel_spmd

    B = x.shape[0]
    H = x.shape[2]
    assert x.shape == (B, C, H, H) and B == N_CORES
    nc = _get_nc((H, 8))
    w = prep_weights(np.asarray(w_off), np.asarray(b_off),
                     np.asarray(w_dcn), np.asarray(b_dcn))
    in_maps = []
    for b in range(B):
        m = dict(w)
        xb = np.asarray(x[b]).astype(ml_dtypes.bfloat16)
        m["x"] = np.pad(xb, ((0, 0), (2, 2), (0, 0)))
        in_maps.append(m)
    return run_bass_kernel_spmd(nc, in_maps, list(range(N_CORES)), **spmd_kwargs)


def kernel(x, w_off, b_off, w_dcn, b_dcn):
    res = _run(x, w_off, b_off, w_dcn, b_dcn)
    out = np.stack([res.results[i]["y"] for i in range(N_CORES)], axis=0)
    return out.astype(np.float32)


# revision 12
# speedup vs baseline: 2.2641x; 1.0169x over previous
"""DCN block kernel for Trainium2 (8 NeuronCores, data-parallel over batch).

Math (per batch b, plane c):
  z   = conv3x3(x, w_off) + b_off                  (64 offset logits)
  s   = sigmoid(z); d = s - 0.5 in (-.5,.5)        (pixel displacement)
  sample at (r - dy, c - dx) bilinear w/ reflect   (|d| < .5 => 3x3 support)
  y   = conv3x3(sampled, w_dcn) + b_dcn

Gather-free sampling via relu-split weights (all DVE ops run in 2x/4x mode):
  A  = relu(sx-.5)  B = relu(.5-sx)   (horizontal taps)
  H_dr = x_dr + A*DL_dr + B*DR_dr     DL = x(c-1)-x(c), DR = x(c+1)-x(c)
  OS = H0 + Av*(Hm-H0) + Bv*(Hp-H0)   (vertical combine)
Reflect at borders: column fixups on DL/DR; row fixups swap the dead Hm/Hp
term at the global top/bottom rows.

Layout: 4 image row-quarters stacked on partition groups [4 x 32ch]; convs
run as 4 concurrent row-tiled matmul streams (tile_position), 9 taps
accumulating in PSUM.

Two schedule tricks:
 - the offset stage for slab i+1 is issued BEFORE the sampling stage of
   slab i, so PE/scalar work overlaps the vector pipeline;
 - slabs are incremental: the 2 sampled halo rows each slab shares with its
   predecessor are copied from the previous OS tile instead of recomputed,
   so conv_off/sigmoid/sampling all run on nr rows, not nr+2.
"""

from contextlib import ExitStack

import ml_dtypes
import numpy as np

import concourse.bacc as bacc
import concourse.bass as bass
import concourse.mybir as mybir
import concourse.tile as tile

BF16 = mybir.dt.bfloat16
F32 = mybir.dt.float32
AF = mybir.ActivationFunctionType
OP = mybir.AluOpType

N_CORES = 8
C = 32          # input/output channels per plane set
OC2 = 64        # offset logits (2 per plane)


class Cfg:
    def __init__(self, H=384, nr=8):
        self.H = H
        self.W = H
        self.WP = self.W + 2          # padded row: [pad, 0..W-1, pad]
        self.QH = H // 4              # rows per quarter
        assert self.QH % nr == 0
        self.nr = nr                  # output rows per quarter per slab
        self.nslab = self.QH // nr


def _f(ap):
    """Flatten free dims of a 3d tile AP to [P, fd]."""
    return ap.rearrange("p a b -> p (a b)")


def build_nc(cfg: Cfg, finalize=True):
    nc = bacc.Bacc()
    H, W, WP, nr = cfg.H, cfg.W, cfg.WP, cfg.nr
    nh = nr + 2   # OS rows:  [r0-1, r0+nr+1)
    nx = nr + 4   # x rows:   [r0-2, r0+nr+2)
    nslab = cfg.nslab

    x_in = nc.declare_dram_parameter("x", [C, H + 4, W], BF16, isOutput=False)
    woff_in = nc.declare_dram_parameter("woff", [128, 9 * OC2], BF16, isOutput=False)
    wdcn_in = nc.declare_dram_parameter("wdcn", [128, 9 * C], BF16, isOutput=False)
    boff_in = nc.declare_dram_parameter("boff", [128, 1], F32, isOutput=False)
    bdcn_in = nc.declare_dram_parameter("bdcn", [128, 1], F32, isOutput=False)
    y_out = nc.declare_dram_parameter("y", [C, H, W], F32, isOutput=True)

    with tile.TileContext(nc) as tc, ExitStack() as ctx:
        consts = ctx.enter_context(tc.tile_pool(name="consts", bufs=1))
        xpool = ctx.enter_context(tc.tile_pool(name="xp", bufs=2))
        spool = ctx.enter_context(tc.tile_pool(name="sp", bufs=1))
        sxpool = ctx.enter_context(tc.tile_pool(name="sxp", bufs=2))
        wpool = ctx.enter_context(tc.tile_pool(name="wp", bufs=1))
        dpool = ctx.enter_context(tc.tile_pool(name="dp", bufs=1))
        hpool = ctx.enter_context(tc.tile_pool(name="hp", bufs=1))
        ospool = ctx.enter_context(tc.tile_pool(name="osp", bufs=2))
        ocpool = ctx.enter_context(tc.tile_pool(name="ocp", bufs=2))
        zpool = ctx.enter_context(tc.tile_pool(name="zp", bufs=2, space="PSUM"))
        opool = ctx.enter_context(tc.tile_pool(name="op", bufs=2, space="PSUM"))

        WOFF = consts.tile([128, 9, OC2], BF16)
        nc.sync.dma_start(out=_f(WOFF), in_=woff_in[:])
        WDCN = consts.tile([128, 9, C], BF16)
        nc.sync.dma_start(out=_f(WDCN), in_=wdcn_in[:])
        BOFF = consts.tile([128, 1], F32)
        nc.sync.dma_start(out=BOFF[:], in_=boff_in[:])
        BDCN = consts.tile([128, 1], F32)
        nc.sync.dma_start(out=BDCN[:], in_=bdcn_in[:])
        PHALF = consts.tile([128, 1], F32)
        nc.vector.memset(PHALF[:], 0.5)

        # persistent x slabs: pre-zero once so pad cols stay zero forever
        XS_list = []
        for sl in range(2):
            XSz = xpool.tile([128, nx, WP], BF16, tag="xs", name=f"xsz{sl}")
            nc.vector.memset(_f(XSz), 0.0)
            XS_list.append(XSz)

        state = {}   # per-slab live tiles, keyed (name, slab)

        def zlo_of(it):
            # slab 0 computes its top halo rows; later slabs copy them from
            # the previous slab's OS tile and only compute OS rows
            # [2, nr+2) == image rows r0+1 .. r0+nr.
            return 0 if it == 0 else 2

        def load_slab(it):
            if it < 2:
                XS = XS_list[it % 2]
            else:
                XS = xpool.tile([128, nx, WP], BF16, tag="xs", name="xs")
            state["xs", it] = XS
            flo = 0 if it == 0 else 2   # steady slabs only read frame rows 2+
            for g in range(4):
                i0 = cfg.QH * g + it * nr   # row in the padded-x frame
                nc.sync.dma_start(
                    out=XS[32 * g:32 * g + 32, flo:nx, 1:W + 1],
                    in_=x_in[:, i0 + flo:i0 + nx, :])

        def stage_offset(it):
            """conv_off + sigmoid + fold + B2/Bv2 acts for slab it."""
            XS = state["xs", it]
            XSf = _f(XS[:])
            zlo = zlo_of(it)
            nz = nr + 2 - zlo
            # conv_off: pairs (q0,q1)->zts[0], (q2,q3)->zts[1]
            SP = spool.tile([128, 2, nz, WP], BF16, tag="sp", name="sp")
            for j in range(nz):
                zz = zpool.tile([128, 2, 512], F32, tag="zz", name="zz")
                for t in range(9):
                    kh, kw = t // 3, t % 3
                    base = (zlo + j + kh) * WP + kw
                    for p in range(2):
                        for gq in range(2):
                            g = 2 * p + gq
                            nc.tensor.matmul(
                                zz[64 * gq:64 * gq + 64, p, 1:W + 1],
                                lhsT=WOFF[32 * g:32 * g + 32, t, :],
                                rhs=XSf[32 * g:32 * g + 32, base:base + W],
                                start=(t == 0), stop=(t == 8),
                                tile_position=(32 * g, 64 * gq),
                                skip_group_check=True)
                nc.scalar.activation(
                    out=SP[:, :, j, 1:W + 1], in_=zz[:, :, 1:W + 1],
                    func=AF.Sigmoid, bias=BOFF[:], scale=1.0)
            # fold s into quarter-stacked SX/SY (8 sbuf->sbuf DMAs); SX first
            # so the sampling stage's A/B2 chain unblocks earliest
            SX = sxpool.tile([128, nz, WP], BF16, tag="sx")
            SY = sxpool.tile([128, nz, WP], BF16, tag="sy")
            for g in range(4):
                p, gq = g // 2, g % 2
                nc.sync.dma_start(
                    out=_f(SX[32 * g:32 * g + 32]),
                    in_=_f(SP[64 * gq:64 * gq + 32, p]))
            for g in range(4):
                p, gq = g // 2, g % 2
                nc.sync.dma_start(
                    out=_f(SY[32 * g:32 * g + 32]),
                    in_=_f(SP[64 * gq + 32:64 * gq + 64, p]))
            state["sx", it] = SX
            state["sy", it] = SY

        def stage_sample(it):
            """DL/DR + A/Av maps + horizontal interps + vertical -> OS."""
            XS = state["xs", it]
            XSf = _f(XS[:])
            zlo = zlo_of(it)
            nz = nr + 2 - zlo
            Lz = nz * WP
            # difference images (no deps beyond XS -> issued first so the
            # vector engine has work while the offset chain finishes)
            L = nx * WP
            DL = dpool.tile([128, nx, WP], BF16, tag="dl")
            DLf = _f(DL)
            nc.vector.tensor_tensor(
                DLf[:, 1:L], XSf[:, 0:L - 1], XSf[:, 1:L], OP.subtract)
            DR = dpool.tile([128, nx, WP], BF16, tag="dr")
            DRf = _f(DR)
            # DRn = x - x(c+1): sign-flipped so its weight can be the
            # vector-computable B2n = min(sx,.5)-.5 = -relu(.5-sx)
            nc.vector.tensor_tensor(
                DRf[:, 0:L - 1], XSf[:, 0:L - 1], XSf[:, 1:L], OP.subtract)
            # reflect fixups at image cols 0 / W-1 (padded cols 1 / W)
            nc.vector.tensor_tensor(
                DL[:, :, 1], DL[:, :, 1], XS[:, :, 2], OP.add)
            nc.vector.tensor_tensor(
                DR[:, :, W], DR[:, :, W], XS[:, :, W - 1], OP.subtract)
            # copy the 2 shared halo rows of OS from the previous slab
            OS = ospool.tile([128, nh, WP], BF16, tag="os")
            if it > 0:
                OSprev = state["os", it - 1]
                nc.vector.tensor_copy(
                    _f(OS[:, 0:2, :]), _f(OSprev[:, nr:nr + 2, :]))
                del state["os", it - 1]
            # weight maps, all on DVE (4x TS):
            #   B2n = min(sx,.5)-.5 = -relu(.5-sx)   (pairs with DRn)
            #   A   = max(sx,.5)-.5 =  relu(sx-.5)   (in place over SX,
            #         emitted after B2n so the read precedes the overwrite)
            SX, SY = state["sx", it], state["sy", it]
            B2 = wpool.tile([128, nz, WP], BF16, tag="b2n")
            nc.vector.tensor_scalar(_f(B2), _f(SX), 0.5, 0.5, OP.min,
                                    OP.subtract)
            nc.vector.tensor_scalar(_f(SX), _f(SX), 0.5, 0.5, OP.max,
                                    OP.subtract)
            A = SX

            # ---- horizontal interps for all three dr in one op each ----
            # plane k of the [128,3,Lz] APs is row-shift dr=k-1; weights are
            # broadcast (stride-0 plane dim).
            def sh3(flat, n):
                b = flat[:, zlo * WP:zlo * WP + Lz]
                lay = [list(b.ap[0]), [WP, n], [1, Lz]]
                return bass.AP(b.tensor, b.offset, lay)

            X3 = sh3(XSf, 3)
            DL3 = sh3(DLf, 3)
            DR3 = sh3(DRf, 3)
            A3 = _f(A).unsqueeze(1).broadcast_to((128, 3, Lz))
            B3 = _f(B2).unsqueeze(1).broadcast_to((128, 3, Lz))
            T1 = hpool.tile([128, 3, nz, WP], BF16, tag="ht1", name="t1all")
            T1c = T1.rearrange("p a b c -> p a (b c)")
            nc.vector.tensor_tensor(T1c, A3, DL3, OP.mult)
            T2 = hpool.tile([128, 3, nz, WP], BF16, tag="ht2", name="t2all")
            T2c = T2.rearrange("p a b c -> p a (b c)")
            nc.vector.tensor_tensor(T2c, B3, DR3, OP.mult)
            nc.vector.tensor_tensor(T1c, T1c, X3, OP.add)
            nc.vector.tensor_tensor(T1c, T1c, T2c, OP.add)
            Hall = T1c          # planes: (Hm, H0, Hp)

            # ---- vertical: OS = H0 + Av*(Hm-H0) + Bv2n*(H0-Hp) ----
            BV2 = wpool.tile([128, 2, nz, WP], BF16, tag="avb")
            BVc = BV2.rearrange("p a b c -> p a (b c)")
            nc.vector.tensor_scalar(BVc[:, 1], _f(SY), 0.5, 0.5, OP.min,
                                    OP.subtract)
            nc.vector.tensor_scalar(BVc[:, 0], _f(SY), 0.5, 0.5, OP.max,
                                    OP.subtract)
            DH = hpool.tile([128, 2, nz, WP], BF16, tag="ht2", name="dh")
            DHc = DH.rearrange("p a b c -> p a (b c)")
            # plane0 = Hm-H0, plane1 = H0-Hp
            nc.vector.tensor_tensor(DHc, Hall[:, 0:2], Hall[:, 1:3],
                                    OP.subtract)
            # T34 in place over DH: plane0 = Av*(Hm-H0), plane1 = Bv2n*(H0-Hp)
            nc.vector.tensor_tensor(DHc, DHc, BVc, OP.mult)
            # reflect fixups at image rows 0 / H-1: recompute the swapped
            # term from the intact Hall planes
            if it == 0:
                nc.vector.tensor_tensor(
                    DHc[0:32, 0, WP:2 * WP], Hall[0:32, 2, WP:2 * WP],
                    Hall[0:32, 1, WP:2 * WP], OP.subtract)
                nc.vector.tensor_tensor(
                    DHc[0:32, 0, WP:2 * WP], DHc[0:32, 0, WP:2 * WP],
                    BVc[0:32, 0, WP:2 * WP], OP.mult)
            if it == nslab - 1:
                jb = nr - zlo   # OS row nr == image row H-1 (quarter 3)
                sl = slice(jb * WP, (jb + 1) * WP)
                nc.vector.tensor_tensor(
                    DHc[96:128, 1, sl], Hall[96:128, 1, sl],
                    Hall[96:128, 0, sl], OP.subtract)
                nc.vector.tensor_tensor(
                    DHc[96:128, 1, sl], DHc[96:128, 1, sl],
                    BVc[96:128, 1, sl], OP.mult)
            OSc = _f(OS[:, zlo:nr + 2, :])
            nc.vector.tensor_tensor(OSc, Hall[:, 1], DHc[:, 0], OP.add)
            nc.vector.tensor_tensor(OSc, OSc, DHc[:, 1], OP.add)
            # sampled outside the image is 0 for the final conv zero-padding
            nc.vector.memset(OS[:, :, 0:WP:W + 1], 0.0)
            if it == 0:
                nc.vector.memset(_f(OS[0:32, 0:1, :]), 0.0)
            if it == nslab - 1:
                nc.vector.memset(_f(OS[96:128, nr + 1:nr + 2, :]), 0.0)
            state["os", it] = OS

        def stage_outconv(it):
            """conv_dcn + bias + store (4-row output chunks)."""
            OSf = _f(state["os", it])
            r0 = it * nr
            OROWS = 4
            for oc_i in range(nr // OROWS):
                OC_t = ocpool.tile([128, OROWS, WP], F32, tag="oc")
                for oj in range(OROWS):
                    oi = oc_i * OROWS + oj
                    ot = opool.tile([128, 512], F32, tag="ot")
                    for t in range(9):
                        kh, kw = t // 3, t % 3
                        base = (oi + kh) * WP + kw
                        for g in range(4):
                            nc.tensor.matmul(
                                ot[32 * g:32 * g + 32, 1:W + 1],
                                lhsT=WDCN[32 * g:32 * g + 32, t, :],
                                rhs=OSf[32 * g:32 * g + 32, base:base + W],
                                start=(t == 0), stop=(t == 8),
                                tile_position=(32 * g, 32 * g),
                                skip_group_check=True)
                    nc.scalar.activation(
                        out=OC_t[:, oj, 1:W + 1], in_=ot[:, 1:W + 1],
                        func=AF.Identity, bias=BDCN[:], scale=1.0)
                for g in range(4):
                    rr = cfg.QH * g + r0 + oc_i * OROWS
                    nc.sync.dma_start(
                        out=y_out[:, rr:rr + OROWS, :],
                        in_=OC_t[32 * g:32 * g + 32, :, 1:W + 1])

        # ---- software-pipelined slab loop ----
        # emission order per iteration: offset(i) | sample(i-1), outconv(i-1)
        # | load(i+1), so the PE's conv_off(i) overlaps the vector work of
        # sample(i-1).
        load_slab(0)
        for it in range(nslab + 1):
            if it < nslab:
                stage_offset(it)
            if it > 0:
                stage_sample(it - 1)
                stage_outconv(it - 1)
            if it + 1 < nslab:
                load_slab(it + 1)
    if finalize:
        nc.finalize()
    return nc


def prep_weights(w_off, b_off, w_dcn, b_dcn):
    """Host-side packing of conv weights into lhsT tiles, replicated x4."""
    perm = np.concatenate([np.arange(0, 2 * C, 2), np.arange(1, 2 * C, 2)])
    # WOFF[32g+ci, kh*3+kw, m] = w_off[perm[m], ci, kh, kw]
    wo = w_off[perm].astype(np.float32)            # [64, C, 3, 3]
    wo = wo.transpose(1, 2, 3, 0).reshape(C, 9, OC2)   # [ci, tap, m]
    woff = np.tile(wo, (4, 1, 1)).reshape(128, 9 * OC2)
    wd = w_dcn.astype(np.float32).transpose(1, 2, 3, 0).reshape(C, 9, C)
    wdcn = np.tile(wd, (4, 1, 1)).reshape(128, 9 * C)
    boff = np.tile(b_off[perm].astype(np.float32), 2).reshape(128, 1)
    bdcn = np.tile(b_dcn.astype(np.float32), 4).reshape(128, 1)
    return {
        "woff": woff.astype(ml_dtypes.bfloat16),
        "wdcn": wdcn.astype(ml_dtypes.bfloat16),
        "boff": boff.astype(np.float32),
        "bdcn": bdcn.astype(np.float32),
    }


_NC_CACHE = {}


def _get_nc(cfg_key):
    if cfg_key not in _NC_CACHE:
        _NC_CACHE[cfg_key] = build_nc(Cfg(H=cfg_key[0], nr=cfg_key[1]))
    return _NC_CACHE[cfg_key]


def _run(x, w_off, b_off, w_dcn, b_dcn, **spmd_kwargs):
    from concourse.bass_utils import run_bass_kernel_spmd

    B = x.shape[0]
    H = x.shape[2]
    assert x.shape == (B, C, H, H) and B == N_CORES
    nc = _get_nc((H, 8))
    w = prep_weights(np.asarray(w_off), np.asarray(b_off),
                     np.asarray(w_dcn), np.asarray(b_dcn))
    in_maps = []
    for b in range(B):
        m = dict(w)
        xb = np.asarray(x[b]).astype(ml_dtypes.bfloat16)
        m["x"] = np.pad(xb, ((0, 0), (2, 2), (0, 0)))
        in_maps.append(m)
    return run_bass_kernel_spmd(nc, in_maps, list(range(N_CORES)), **spmd_kwargs)


def kernel(x, w_off, b_off, w_dcn, b_dcn):
    res = _run(x, w_off, b_off, w_dcn, b_dcn)
    out = np.stack([res.results[i]["y"] for i in range(N_CORES)], axis=0)
    return out.astype(np.float32)
